# revision 46
# baseline (speedup 1.0000x reference)
"""Trainium2 Bass kernel for nn_Mlp_moe: dense patch-token MLP + top-1 gated
atom (expert) routing for 6 CLS task tokens.

Sharding over 8 NeuronCores:
  - Patch MLP: data-parallel over batch B=64 -> 8 batches (1568 patch tokens)
    per core. MLP weights replicated (SBUF-resident, bf16).
  - Atom/CLS part: hidden dim H=3072 sharded 8-way (384 per core); every core
    processes all 384 CLS tokens on its H-shard and emits a partial output
    summed on the host. Routing (gate logits/sigmoid/top-1) is computed on
    the host (O(B*6*D), negligible) and shipped as mask vectors.

Structure of the routing: token at slot n either goes left (src=n//2,
dst=3+n%2) or right (src=3+n%2, dst=n//2) with weight w. The in-GEMM is
computed for all 5 atoms (it doubles as DMA-latency cover at kernel start);
the chosen hid is selected with masks; the out-GEMM only computes the two
dst candidates per token (2/5 of the all-atom cost):
  psumL[tok in slots 2a,2a+1] += aout[a]    @ (sel * w*1[right])   a=0,1,2
  psumH[tok, parity j blocks] += aout[3+j]  @ (sel * w*1[left])    j=0,1

DMA: inputs are packed host-side into partition-major slices and issued in
compute-need order, alternating between the two fast HWDGE queues (sync /
scalar); per-(a,k) ain slices and per-4-h-tile w1/w2 slices let compute
start as soon as the first ~1MB lands instead of after whole tensors.
Patch outputs are written in bf16 (host upcasts and adds the bias).
"""

import numpy as np
import ml_dtypes

import concourse.bass as bass
import concourse.bacc as bacc
import concourse.mybir as mybir
from concourse import tile
from concourse.bass_utils import run_bass_kernel_spmd

NCORES = 8
B, NCLS, P, D, H = 64, 6, 196, 768, 3072
NA = 5
HSH = H // NCORES            # 384: per-core atom hidden shard
BPC = B // NCORES            # 8 batches per core
TPC = BPC * P                # 1568 patch tokens per core
NT = B * NCLS                # 384 cls tokens
DT = D // 128                # 6 d-tiles
HT = H // 128                # 24 h-tiles
KPA = HSH // 128             # 3 h-shard tiles per atom
NG = NA * KPA                # 15 (atom, k) in-GEMM groups
CW = 392
NCH = 4
WP = 4                       # h-tiles per w1/w2 DMA piece
NWP = HT // WP               # 6 pieces

LEFT_KEYS = np.array([3, 4, 8, 9, 13, 14], dtype=np.int64)
RIGHT_KEYS = np.array([15, 20, 16, 21, 17, 22], dtype=np.int64)

BF16 = mybir.dt.bfloat16
F32 = mybir.dt.float32
AF = mybir.ActivationFunctionType

_CACHE = {}
LAST_RESULTS = None  # BassKernelResults of the most recent run (for profiling)


def _build_program():
    nc = bacc.Bacc(None, target_bir_lowering=False, debug=False,
                   num_devices=NCORES)

    # ---- DRAM inputs (partition-major packed, see host layouts below) ----
    clsT_d = nc.dram_tensor("clsT", [128, DT * NT], BF16,
                            kind="ExternalInput")
    ainbT_d = nc.dram_tensor("ainbT", [128, NG], F32, kind="ExternalInput")
    b1T_d = nc.dram_tensor("b1T", [128, HT], F32, kind="ExternalInput")
    ain2_d = nc.dram_tensor("ain2", [NG, 128, DT * 128], BF16,
                            kind="ExternalInput")
    xT_d = nc.dram_tensor("xT", [128, NCH * DT * CW], BF16,
                          kind="ExternalInput")
    w1s_d = nc.dram_tensor("w1s", [128, HT * DT * 128], BF16,
                           kind="ExternalInput")
    w2T_d = nc.dram_tensor("w2T", [128, HT * D], BF16, kind="ExternalInput")
    mall_d = nc.dram_tensor("mall", [128, (NA + 2) * NT], BF16,
                            kind="ExternalInput")
    aoutF_d = nc.dram_tensor("aoutF", [128, NA * KPA * D], BF16,
                             kind="ExternalInput")
    # outputs: pout2[ci][p, d*CW + t] = patch_out_pre_bias[d*128+p, ci*CW+t]
    pout2_d = nc.dram_tensor("pout2", [NCH, 128, DT * CW], BF16,
                             kind="ExternalOutput")
    cpartT_d = nc.dram_tensor("cpartT", [128, DT * NT], F32,
                              kind="ExternalOutput")

    with tile.TileContext(nc) as tc:
        with (
            tc.tile_pool(name="w", bufs=1) as wp,
            tc.tile_pool(name="gat", bufs=1) as gp,
            tc.tile_pool(name="sel", bufs=1) as sp,
            tc.tile_pool(name="g1", bufs=24) as g1p,
            tc.tile_pool(name="ostg", bufs=1) as op,
            tc.tile_pool(name="cstg", bufs=1) as cp,
            tc.tile_pool(name="ps", bufs=8, space="PSUM") as pp,
        ):
            # ---- resident tiles ----
            # cls declared [128, DT, NCLS, 64] so phase A can take stepped
            # slot slices as matmul moving operands
            clsT = wp.tile([128, DT, NCLS, 64], BF16, tag="cls", name="cls")
            ainbT = wp.tile([128, NG], F32, tag="ainb", name="ainb")
            b1T = wp.tile([128, HT], F32, tag="b1", name="b1")
            ain2 = [wp.tile([128, DT * 128], BF16, tag=f"ain{g}",
                            name=f"ain{g}") for g in range(NG)]
            xs = [wp.tile([128, DT * CW], BF16, tag=f"x{c}", name=f"x{c}")
                  for c in range(NCH)]
            w1s = wp.tile([128, HT * DT * 128], BF16, tag="w1", name="w1")
            w2T = wp.tile([128, HT * D], BF16, tag="w2", name="w2")
            # masks combined into one tile: cols [0, NA*NT) = msrc per atom,
            # [NA*NT, (NA+1)*NT) = mL, [(NA+1)*NT, (NA+2)*NT) = mR
            mall = wp.tile([128, (NA + 2) * NT], BF16, tag="mall",
                           name="mall")
            aoutF = wp.tile([128, NA * KPA * D], BF16, tag="aoF",
                            name="aoF")

            # ---- DMA emission machinery ----
            # dma_start issues each wait on one of 8 round-robin completion
            # semaphore lanes (issue N blocks on completion of issue N-8), so
            # a long run of back-to-back dma_starts serializes the issuing
            # engine's queue.  Only the startup-critical transfers are issued
            # up front; the rest are threaded into the compute stream at
            # points where their lane-waits have long resolved.  Alternating
            # sync/scalar (the two HWDGE rings) keeps both rings pulling, and
            # FIFO-per-ring makes data land in need order.
            XW = DT * CW
            W1P = WP * DT * 128
            W2P = WP * D
            items = {}
            for j in range(3):  # cls in d-pair thirds
                c0, c1 = j * 2 * NT, (j + 1) * 2 * NT
                items[f"cls{j}"] = (clsT[:, 2 * j:2 * (j + 1), :, :],
                                    clsT_d[:, c0:c1])
            items["ainb"] = (ainbT[:], ainbT_d[:])
            items["b1"] = (b1T[:], b1T_d[:])
            for g in range(NG):
                items[f"ain{g}"] = (ain2[g][:], ain2_d[g])
            for hf in range(2):
                c0 = hf * (XW // 2)
                items[f"x0_{hf}"] = (xs[0][:, c0:c0 + XW // 2],
                                     xT_d[:, c0:c0 + XW // 2])
            for c in range(1, NCH):
                items[f"x{c}"] = (xs[c][:], xT_d[:, c * XW:(c + 1) * XW])
            W1H = DT * 128  # w1 cols per h-tile
            W1CUTS = [0, 2, 6, 12, 18, 24]  # first slices finer: GEMM1 can
            for j in range(5):              # start as soon as h0-1 land
                c0, c1 = W1CUTS[j] * W1H, W1CUTS[j + 1] * W1H
                items[f"w1p{j}"] = (w1s[:, c0:c1], w1s_d[:, c0:c1])
            for j in range(2):  # w2 in halves
                c0, c1 = j * 12 * D, (j + 1) * 12 * D
                items[f"w2p{j}"] = (w2T[:, c0:c1], w2T_d[:, c0:c1])
            items["mall"] = (mall[:], mall_d[:])
            AOH = NA * KPA * D // 2
            for j in range(2):  # aout in halves
                c0, c1 = j * AOH, (j + 1) * AOH if j else AOH
                items[f"aoF{j}"] = (aoutF[:, c0:c1], aoutF_d[:, c0:c1])

            dma_i = [0]

            def dma(*names, eng=None):
                for nm in names:
                    dst, src = items.pop(nm)
                    e = eng
                    if e is None:
                        e = nc.sync if dma_i[0] % 2 == 0 else nc.scalar
                        dma_i[0] += 1
                    e.dma_start(dst, src)

            # scalar (which also runs the gelus) gets ONLY the small
            # startup-critical transfers; its dma-issue chain ends by ~16us.
            # sync gets everything else up front in need order: its
            # semaphore-lane waits serialize harmlessly (no compute there),
            # and a lone HWDGE ring saturates HBM (~350GB/s measured).
            # x1..x3 ride at the end of the sync chain: the lane
            # serialization delays their issue past the startup-critical
            # window, so they never compete with cls/ain/w1 for HBM.
            dma("cls1", *[f"ain{g}" for g in range(0, NG, 2)],
                eng=nc.scalar)
            dma("cls0", "cls2", "ainb", "b1",
                *[f"ain{g}" for g in range(1, NG, 2)],
                "x0_0", "x0_1", *[f"w1p{j}" for j in range(5)],
                "w2p0", "w2p1", "mall", "aoF0", "aoF1",
                "x1", "x2", "x3",
                eng=nc.sync)

            # ---- PE warm-up: dummy matmuls while the first DMAs land, so
            # the HAM clock-gate reaches 8/8 before real work starts ----
            warm = wp.tile([128, 512], BF16, tag="warm", name="warm")
            nc.vector.memset(warm[:], 0)
            wps = pp.tile([128, 512], F32, tag="ps", name="ps")
            NWARM = 9  # >=3.4us of PE busy at the cold clock flips HAM warm
            for i in range(NWARM):
                nc.tensor.matmul(wps[:, :512], warm[:, :128], warm[:],
                                 start=(i == 0), stop=(i == NWARM - 1))

            # ---- phase A: atom in-GEMM + gelu, candidate tokens only ----
            # atom a<3 is a left-src candidate for slots {2a, 2a+1} (128
            # tokens, contiguous in slot-major cls); atoms 3,4 are right-src
            # candidates for parity (a-3) slots (192 tokens, stepped slice)
            G = {}
            for a in range(NA):
                for k in range(KPA):
                    g = a * KPA + k
                    wv = 128 if a < 3 else 192
                    ps = pp.tile([128, 512], F32, tag="ps", name="ps")
                    for d in range(DT):
                        if a < 3:
                            mv = clsT[:, d, 2 * a:2 * a + 2, :]
                        else:
                            mv = clsT[:, d, (a - 3)::2, :]
                        nc.tensor.matmul(ps[:, :wv],
                                         ain2[g][:, d * 128:(d + 1) * 128],
                                         mv, start=(d == 0),
                                         stop=(d == DT - 1))
                    gt = gp.tile([128, wv], BF16, tag=f"g{g}", name=f"g{g}")
                    nc.scalar.activation(gt[:], ps[:, :wv], AF.Gelu,
                                         bias=ainbT[:, g:g + 1])
                    G[g] = gt

            def phase_b():
                # ---- phase B (DVE): apply the routing masks directly.
                # Left-choosers: hid comes from the low-atom candidate G,
                # goes to high dst -> hidH; right-choosers: hid from the
                # high-atom candidate G, goes to low dst -> hidL.
                # hidH[:, 192j+64i+b] = G[(i,k)][:, 64j+b]   * mL[n=2i+j]
                # hidL[:, 128a+64p+b] = G[(3+p,k)][:, 64a+b] * mR[n=2a+p]
                hidL, hidH = [], []
                for k in range(KPA):
                    hl = sp.tile([128, NT], BF16, tag=f"hL{k}",
                                 name=f"hL{k}")
                    hh = sp.tile([128, NT], BF16, tag=f"hH{k}",
                                 name=f"hH{k}")
                    for j in range(2):
                        for i in range(3):
                            dc = 192 * j + 64 * i
                            mc = NA * NT + (2 * i + j) * 64
                            nc.vector.tensor_mul(
                                hh[:, dc:dc + 64],
                                G[i * KPA + k][:, 64 * j:64 * j + 64],
                                mall[:, mc:mc + 64])
                    for p in range(2):
                        for a in range(3):
                            dc = 128 * a + 64 * p
                            mc = (NA + 1) * NT + (2 * a + p) * 64
                            nc.vector.tensor_mul(
                                hl[:, dc:dc + 64],
                                G[(3 + p) * KPA + k][:, 64 * a:64 * a + 64],
                                mall[:, mc:mc + 64])
                    hidL.append(hl)
                    hidH.append(hh)
                return hidL, hidH

            def patch_chunk(ci, mid=None, g2_split=False):
                xa = xs[ci]
                g1s = []
                for h in range(HT):
                    ps = pp.tile([128, 512], F32, tag="ps", name="ps")
                    for d in range(DT):
                        nc.tensor.matmul(
                            ps[:, :CW],
                            w1s[:, h * DT * 128 + d * 128:
                                h * DT * 128 + (d + 1) * 128],
                            xa[:, d * CW:(d + 1) * CW],
                            start=(d == 0), stop=(d == DT - 1))
                    g1 = g1p.tile([128, CW], BF16, tag="g1", name="g1")
                    nc.scalar.activation(g1[:], ps[:, :CW], AF.Gelu,
                                         bias=b1T[:, h:h + 1])
                    g1s.append(g1)
                mid_out = mid() if mid is not None else None
                stg = op.tile([128, DT * CW], BF16, tag="ostg", name="ostg")
                if g2_split:
                    # two h-passes over all dp banks: h16-23's w2 slices
                    # (still in flight at chunk-0 time) are only needed for
                    # the second pass, ~16us later than dp-major order
                    psd = [pp.tile([128, 512], F32, tag="ps", name="ps")
                           for _ in range(DT)]
                    for h0, h1 in ((0, 16), (16, HT)):
                        for dp in range(DT):
                            for h in range(h0, h1):
                                nc.tensor.matmul(
                                    psd[dp][:, :CW],
                                    w2T[:, h * D + dp * 128:
                                        h * D + (dp + 1) * 128],
                                    g1s[h][:, :CW],
                                    start=(h == 0), stop=(h == HT - 1),
                                    skip_group_check=True)
                    for dp in range(DT):
                        nc.vector.tensor_copy(stg[:, dp * CW:(dp + 1) * CW],
                                              psd[dp][:, :CW])
                else:
                    for dp in range(DT):
                        ps = pp.tile([128, 512], F32, tag="ps", name="ps")
                        for h in range(HT):
                            nc.tensor.matmul(
                                ps[:, :CW],
                                w2T[:, h * D + dp * 128:h * D + (dp + 1) * 128],
                                g1s[h][:, :CW],
                                start=(h == 0), stop=(h == HT - 1))
                        nc.vector.tensor_copy(stg[:, dp * CW:(dp + 1) * CW],
                                              ps[:, :CW])
                        if ci == NCH - 1:
                            # tail chunk: per-dp DMAs on the fast queues
                            eng = nc.sync if dp % 2 == 0 else nc.scalar
                            eng.dma_start(
                                pout2_d[ci][:, dp * CW:(dp + 1) * CW],
                                stg[:, dp * CW:(dp + 1) * CW])
                if ci != NCH - 1:
                    nc.gpsimd.dma_start(pout2_d[ci], stg[:])
                return mid_out

            # ---- patch chunk 0, then atom out-GEMM, then chunks 1..3 ----
            hidL, hidH = patch_chunk(0, mid=phase_b, g2_split=True)

            cstg = cp.tile([128, DT * NT], F32, tag="cstg", name="cstg")
            for dp in range(DT):
                psL = pp.tile([128, 512], F32, tag="ps", name="ps")
                n = 0
                for a in range(3):
                    for k in range(KPA):
                        c0 = a * KPA * D + k * D + dp * 128
                        nc.tensor.matmul(
                            psL[:, a * 128:(a + 1) * 128],
                            aoutF[:, c0:c0 + 128],
                            hidL[k][:, a * 128:(a + 1) * 128],
                            start=(n == 0), stop=(n == 3 * KPA - 1))
                        n += 1
                psH = pp.tile([128, 512], F32, tag="ps", name="ps")
                n = 0
                for j in range(2):
                    for k in range(KPA):
                        c0 = (3 + j) * KPA * D + k * D + dp * 128
                        nc.tensor.matmul(
                            psH[:, j * 192:(j + 1) * 192],
                            aoutF[:, c0:c0 + 128],
                            hidH[k][:, j * 192:(j + 1) * 192],
                            start=(n == 0), stop=(n == 2 * KPA - 1))
                        n += 1
                # merge: cstg[slot-major] = psL[slot-major] + psH[parity]
                # (DVE may read only one operand from PSUM: copy, then add)
                nc.vector.tensor_copy(cstg[:, dp * NT:(dp + 1) * NT],
                                      psL[:, :NT])
                for nn in range(NCLS):
                    scol = 192 * (nn % 2) + 64 * (nn // 2)
                    cs = cstg[:, dp * NT + nn * 64:dp * NT + (nn + 1) * 64]
                    nc.vector.tensor_add(cs, cs, psH[:, scol:scol + 64])
            nc.sync.dma_start(cpartT_d[:], cstg[:])

            patch_chunk(1)
            patch_chunk(2)
            patch_chunk(3)

    nc.compile()
    return nc


def _sigmoid(x):
    out = np.empty_like(x)
    pos = x >= 0
    out[pos] = 1.0 / (1.0 + np.exp(-x[pos]))
    ex = np.exp(x[~pos])
    out[~pos] = ex / (1.0 + ex)
    return out


def kernel(x, patch_w1, patch_b1, patch_w2, patch_b2, gate_delta,
           atom_in_w, atom_in_b, atom_out_w, atom_out_b):
    x = np.asarray(x, dtype=np.float32)
    patch_w1 = np.asarray(patch_w1, dtype=np.float32)
    patch_b1 = np.asarray(patch_b1, dtype=np.float32)
    patch_w2 = np.asarray(patch_w2, dtype=np.float32)
    patch_b2 = np.asarray(patch_b2, dtype=np.float32)
    gate_delta = np.asarray(gate_delta, dtype=np.float32)
    atom_in_w = np.asarray(atom_in_w, dtype=np.float32)
    atom_in_b = np.asarray(atom_in_b, dtype=np.float32)
    atom_out_w = np.asarray(atom_out_w, dtype=np.float32)
    atom_out_b = np.asarray(atom_out_b, dtype=np.float32)

    bf = ml_dtypes.bfloat16

    # ---- host routing (tiny) ----
    cls3 = x[:, :NCLS, :]                                   # [B, 6, D]
    logits = np.einsum("bnd,nd->bn", cls3, gate_delta)      # [B, 6] f32
    choose_left = logits >= 0
    p_left = _sigmoid(logits)
    wgt = np.where(choose_left, p_left, 1.0 - p_left).astype(np.float32)
    keys = np.where(choose_left, LEFT_KEYS[None, :], RIGHT_KEYS[None, :])
    dst = (keys % NA).reshape(-1)                           # [B*6]
    wflat = wgt.reshape(-1)

    # token order on device: t = n*64 + b (slot-major)
    src_sm = (keys // NA).T.reshape(-1)                     # [384] slot-major
    left_sm = choose_left.T.reshape(-1)
    w_sm = wgt.T.reshape(-1)

    msrc = (src_sm[None, :] == np.arange(NA)[:, None]).astype(np.float32)
    mL = np.where(left_sm, w_sm, 0.0).astype(np.float32)
    mR = np.where(~left_sm, w_sm, 0.0).astype(np.float32)
    mall_rep = np.ascontiguousarray(np.broadcast_to(
        np.concatenate([msrc.reshape(-1), mL, mR]).reshape(1, (NA + 2) * NT),
        (128, (NA + 2) * NT))).astype(bf)

    # ---- replicated tensors (partition-major packed) ----
    # clsT[p, d*NT + t] = cls_sm[t, d*128+p],  t slot-major (n*64+b)
    cls_sm = np.ascontiguousarray(cls3.transpose(1, 0, 2)).reshape(NT, D)
    clsT = np.ascontiguousarray(
        cls_sm.reshape(NT, DT, 128).transpose(2, 1, 0)
    ).reshape(128, DT * NT).astype(bf)
    # w1s[p, h*768 + d*128 + m] = patch_w1[h*128+m, d*128+p]
    w1s = np.ascontiguousarray(
        patch_w1.reshape(HT, 128, DT, 128).transpose(3, 0, 2, 1)
    ).reshape(128, HT * DT * 128).astype(bf)
    b1T = np.ascontiguousarray(patch_b1.reshape(HT, 128).T)
    # w2T[p, h*D + dp*128 + m] = patch_w2[dp*128+m, h*128+p]
    w2T = np.ascontiguousarray(
        patch_w2.reshape(DT, 128, HT, 128).transpose(3, 2, 0, 1)
    ).reshape(128, HT * D).astype(bf)

    # ---- per-core tensors ----
    patch = x[:, NCLS:, :].reshape(NCORES, TPC, D)
    # xT[p, ci*DT*CW + d*CW + t] = patch[c][ci*CW+t, d*128+p]
    xT_all = np.ascontiguousarray(
        patch.reshape(NCORES, NCH, CW, DT, 128).transpose(0, 4, 1, 3, 2)
    ).reshape(NCORES, 128, NCH * DT * CW).astype(bf)

    ain2_all, ainbT_all, aoutT_all = [], [], []
    for c in range(NCORES):
        hsl = slice(HSH * c, HSH * (c + 1))
        # ain2[a*3+k][p, d*128+m] = atom_in_w[a, hsl0+k*128+m, d*128+p]
        ain2 = np.ascontiguousarray(
            atom_in_w[:, hsl, :].reshape(NA, KPA, 128, DT, 128)
            .transpose(0, 1, 4, 3, 2)).reshape(NG, 128, DT * 128).astype(bf)
        ain2_all.append(ain2)
        ainbT_all.append(np.ascontiguousarray(
            atom_in_b[:, hsl].reshape(NG, 128).T))
        # aoutF[p, a*KPA*D + k*D + dp*128 + m]
        #   = atom_out_w[a, dp*128+m, hsl0+k*128+p]
        aoutF = np.ascontiguousarray(
            atom_out_w[:, :, hsl].reshape(NA, DT, 128, KPA, 128)
            .transpose(4, 0, 3, 1, 2)).reshape(128, NA * KPA * D).astype(bf)
        aoutT_all.append(aoutF)

    in_maps = []
    for c in range(NCORES):
        in_maps.append({
            "xT": xT_all[c], "w1s": w1s, "b1T": b1T, "w2T": w2T,
            "clsT": clsT, "ain2": ain2_all[c], "ainbT": ainbT_all[c],
            "aoutF": aoutT_all[c], "mall": mall_rep,
        })

    nc = _CACHE.get("nc")
    if nc is None:
        nc = _build_program()
        _CACHE["nc"] = nc

    res = run_bass_kernel_spmd(nc, in_maps, core_ids=list(range(NCORES)))
    global LAST_RESULTS
    LAST_RESULTS = res

    # ---- host gather ----
    patch_out = np.empty((B, P, D), dtype=np.float32)
    for c in range(NCORES):
        # pout2[ci][p, d*CW+t] -> [D, TPC]
        po = res.results[c]["pout2"].astype(np.float32)     # [NCH,128,DT*CW]
        po = po.reshape(NCH, 128, DT, CW).transpose(2, 1, 0, 3).reshape(D, TPC)
        patch_out[BPC * c:BPC * (c + 1)] = (
            po.T + patch_b2[None, :]).reshape(BPC, P, D)

    cpart = np.zeros((128, DT * NT), dtype=np.float32)
    for c in range(NCORES):
        cpart += res.results[c]["cpartT"]
    cls_sm_out = cpart.reshape(128, DT, NT).transpose(1, 0, 2).reshape(D, NT).T
    # slot-major [n*64+b] -> [b, n]
    cls_out = np.ascontiguousarray(
        cls_sm_out.reshape(NCLS, B, D).transpose(1, 0, 2))
    cls_out += (wflat[:, None] * atom_out_b[dst, :]).reshape(B, NCLS, D)

    return np.concatenate([cls_out, patch_out], axis=1)


# revision 47
# speedup vs baseline: 1.1515x; 1.1515x over previous
"""Trainium2 Bass kernel for nn_Mlp_moe: dense patch-token MLP + top-1 gated
atom (expert) routing for 6 CLS task tokens.

Sharding over 8 NeuronCores:
  - Patch MLP: data-parallel over batch B=64 -> 8 batches (1568 patch tokens)
    per core. MLP weights replicated (SBUF-resident, bf16).
  - Atom/CLS part: hidden dim H=3072 sharded 8-way (384 per core); every core
    processes all 384 CLS tokens on its H-shard and emits a partial output
    summed on the host. Routing (gate logits/sigmoid/top-1) is computed on
    the host (O(B*6*D), negligible) and shipped as mask vectors.

Structure of the routing: token at slot n either goes left (src=n//2,
dst=3+n%2) or right (src=3+n%2, dst=n//2) with weight w. The in-GEMM is
computed for all 5 atoms (it doubles as DMA-latency cover at kernel start);
the chosen hid is selected with masks; the out-GEMM only computes the two
dst candidates per token (2/5 of the all-atom cost):
  psumL[tok in slots 2a,2a+1] += aout[a]    @ (sel * w*1[right])   a=0,1,2
  psumH[tok, parity j blocks] += aout[3+j]  @ (sel * w*1[left])    j=0,1

DMA: inputs are packed host-side into partition-major slices and issued in
compute-need order, alternating between the two fast HWDGE queues (sync /
scalar); per-(a,k) ain slices and per-4-h-tile w1/w2 slices let compute
start as soon as the first ~1MB lands instead of after whole tensors.
Patch outputs are written in bf16 (host upcasts and adds the bias).
"""

import numpy as np
import ml_dtypes

import concourse.bass as bass
import concourse.bacc as bacc
import concourse.mybir as mybir
from concourse import tile
from concourse.bass_utils import run_bass_kernel_spmd

NCORES = 8
B, NCLS, P, D, H = 64, 6, 196, 768, 3072
NA = 5
HSH = H // NCORES            # 384: per-core atom hidden shard
BPC = B // NCORES            # 8 batches per core
TPC = BPC * P                # 1568 patch tokens per core
NT = B * NCLS                # 384 cls tokens
DT = D // 128                # 6 d-tiles
HT = H // 128                # 24 h-tiles
KPA = HSH // 128             # 3 h-shard tiles per atom
NG = NA * KPA                # 15 (atom, k) in-GEMM groups
CW = 392
NCH = 4
WP = 4                       # h-tiles per w1/w2 DMA piece
NWP = HT // WP               # 6 pieces

LEFT_KEYS = np.array([3, 4, 8, 9, 13, 14], dtype=np.int64)
RIGHT_KEYS = np.array([15, 20, 16, 21, 17, 22], dtype=np.int64)

BF16 = mybir.dt.bfloat16
F32 = mybir.dt.float32
AF = mybir.ActivationFunctionType

_CACHE = {}
LAST_RESULTS = None  # BassKernelResults of the most recent run (for profiling)


def _build_program():
    nc = bacc.Bacc(None, target_bir_lowering=False, debug=False,
                   num_devices=NCORES)

    # ---- DRAM inputs (partition-major packed, see host layouts below) ----
    clsT_d = nc.dram_tensor("clsT", [128, DT * NT], BF16,
                            kind="ExternalInput")
    ainbT_d = nc.dram_tensor("ainbT", [128, NG], F32, kind="ExternalInput")
    b1T_d = nc.dram_tensor("b1T", [128, HT], F32, kind="ExternalInput")
    ain2_d = nc.dram_tensor("ain2", [NG, 128, DT * 128], BF16,
                            kind="ExternalInput")
    xT_d = nc.dram_tensor("xT", [128, NCH * DT * CW], BF16,
                          kind="ExternalInput")
    w1s_d = nc.dram_tensor("w1s", [128, HT * DT * 128], BF16,
                           kind="ExternalInput")
    w2T_d = nc.dram_tensor("w2T", [128, HT * D], BF16, kind="ExternalInput")
    mall_d = nc.dram_tensor("mall", [128, (NA + 2) * NT], BF16,
                            kind="ExternalInput")
    aoutF_d = nc.dram_tensor("aoutF", [128, NA * KPA * D], BF16,
                             kind="ExternalInput")
    # outputs: pout2[ci][p, d*CW + t] = patch_out_pre_bias[d*128+p, ci*CW+t]
    pout2_d = nc.dram_tensor("pout2", [NCH, 128, DT * CW], BF16,
                             kind="ExternalOutput")
    cpartT_d = nc.dram_tensor("cpartT", [128, DT * NT], F32,
                              kind="ExternalOutput")

    with tile.TileContext(nc) as tc:
        with (
            tc.tile_pool(name="w", bufs=1) as wp,
            tc.tile_pool(name="gat", bufs=1) as gp,
            tc.tile_pool(name="sel", bufs=1) as sp,
            tc.tile_pool(name="g1", bufs=24) as g1p,
            tc.tile_pool(name="ostg", bufs=1) as op,
            tc.tile_pool(name="cstg", bufs=1) as cp,
            tc.tile_pool(name="ps", bufs=8, space="PSUM") as pp,
        ):
            # ---- resident tiles ----
            clsT = wp.tile([128, DT * NT], BF16, tag="cls", name="cls")
            ainbT = wp.tile([128, NG], F32, tag="ainb", name="ainb")
            b1T = wp.tile([128, HT], F32, tag="b1", name="b1")
            ain2 = [wp.tile([128, DT * 128], BF16, tag=f"ain{g}",
                            name=f"ain{g}") for g in range(NG)]
            xs = [wp.tile([128, DT * CW], BF16, tag=f"x{c}", name=f"x{c}")
                  for c in range(NCH)]
            w1s = wp.tile([128, HT * DT * 128], BF16, tag="w1", name="w1")
            w2T = wp.tile([128, HT * D], BF16, tag="w2", name="w2")
            # masks combined into one tile: cols [0, NA*NT) = msrc per atom,
            # [NA*NT, (NA+1)*NT) = mL, [(NA+1)*NT, (NA+2)*NT) = mR
            mall = wp.tile([128, (NA + 2) * NT], BF16, tag="mall",
                           name="mall")
            aoutF = wp.tile([128, NA * KPA * D], BF16, tag="aoF",
                            name="aoF")

            # ---- DMA emission machinery ----
            # dma_start issues each wait on one of 8 round-robin completion
            # semaphore lanes (issue N blocks on completion of issue N-8), so
            # a long run of back-to-back dma_starts serializes the issuing
            # engine's queue.  Only the startup-critical transfers are issued
            # up front; the rest are threaded into the compute stream at
            # points where their lane-waits have long resolved.  Alternating
            # sync/scalar (the two HWDGE rings) keeps both rings pulling, and
            # FIFO-per-ring makes data land in need order.
            XW = DT * CW
            W1P = WP * DT * 128
            W2P = WP * D
            items = {}
            for j in range(3):  # cls in d-pair thirds
                c0, c1 = j * 2 * NT, (j + 1) * 2 * NT
                items[f"cls{j}"] = (clsT[:, c0:c1], clsT_d[:, c0:c1])
            items["ainb"] = (ainbT[:], ainbT_d[:])
            items["b1"] = (b1T[:], b1T_d[:])
            for g in range(NG):
                items[f"ain{g}"] = (ain2[g][:], ain2_d[g])
            for hf in range(2):
                c0 = hf * (XW // 2)
                items[f"x0_{hf}"] = (xs[0][:, c0:c0 + XW // 2],
                                     xT_d[:, c0:c0 + XW // 2])
            for c in range(1, NCH):
                items[f"x{c}"] = (xs[c][:], xT_d[:, c * XW:(c + 1) * XW])
            for j in range(NWP):
                items[f"w1p{j}"] = (w1s[:, j * W1P:(j + 1) * W1P],
                                    w1s_d[:, j * W1P:(j + 1) * W1P])
            for j in range(2):  # w2 in halves
                c0, c1 = j * 12 * D, (j + 1) * 12 * D
                items[f"w2p{j}"] = (w2T[:, c0:c1], w2T_d[:, c0:c1])
            items["mall"] = (mall[:], mall_d[:])
            AOH = NA * KPA * D // 2
            for j in range(2):  # aout in halves
                c0, c1 = j * AOH, (j + 1) * AOH if j else AOH
                items[f"aoF{j}"] = (aoutF[:, c0:c1], aoutF_d[:, c0:c1])

            dma_i = [0]

            def dma(*names, eng=None):
                for nm in names:
                    dst, src = items.pop(nm)
                    e = eng
                    if e is None:
                        e = nc.sync if dma_i[0] % 2 == 0 else nc.scalar
                        dma_i[0] += 1
                    e.dma_start(dst, src)

            # scalar (which also runs the gelus) gets ONLY the small
            # startup-critical transfers; its dma-issue chain ends by ~16us.
            # sync gets everything else up front in need order: its
            # semaphore-lane waits serialize harmlessly (no compute there),
            # and a lone HWDGE ring saturates HBM (~350GB/s measured).
            # x1..x3 ride at the end of the sync chain: the lane
            # serialization delays their issue past the startup-critical
            # window, so they never compete with cls/ain/w1 for HBM.
            dma("cls1", *[f"ain{g}" for g in range(0, NG, 2)],
                eng=nc.scalar)
            dma("cls0", "cls2", "ainb", "b1",
                *[f"ain{g}" for g in range(1, NG, 2)],
                "x0_0", "x0_1", *[f"w1p{j}" for j in range(NWP)],
                "w2p0", "w2p1", "mall", "aoF0", "aoF1",
                "x1", "x2", "x3",
                eng=nc.sync)

            # ---- PE warm-up: dummy matmuls while the first DMAs land, so
            # the HAM clock-gate reaches 8/8 before real work starts ----
            warm = wp.tile([128, 512], BF16, tag="warm", name="warm")
            nc.vector.memset(warm[:], 0)
            wps = pp.tile([128, 512], F32, tag="ps", name="ps")
            NWARM = 9  # >=3.4us of PE busy at the cold clock flips HAM warm
            for i in range(NWARM):
                nc.tensor.matmul(wps[:, :512], warm[:, :128], warm[:],
                                 start=(i == 0), stop=(i == NWARM - 1))

            # ---- phase A: atom in-GEMM + gelu, all 5 atoms, (a,k)-major ---
            G = {}
            for g in range(NG):
                ps = pp.tile([128, 512], F32, tag="ps", name="ps")
                for d in range(DT):
                    nc.tensor.matmul(ps[:, :NT],
                                     ain2[g][:, d * 128:(d + 1) * 128],
                                     clsT[:, d * NT:(d + 1) * NT],
                                     start=(d == 0), stop=(d == DT - 1))
                gt = gp.tile([128, NT], BF16, tag=f"g{g}", name=f"g{g}")
                nc.scalar.activation(gt[:], ps[:, :NT], AF.Gelu,
                                     bias=ainbT[:, g:g + 1])
                G[g] = gt

            def phase_b():
                # ---- phase B (DVE): src-select, then dst-candidate masks
                hidL, hidH = [], []
                for k in range(KPA):
                    sel = sp.tile([128, NT], BF16, tag=f"sel{k}",
                                  name=f"sel{k}")
                    tmp = sp.tile([128, NT], BF16, tag="tmp", name="tmp")
                    nc.vector.tensor_mul(sel[:], G[0 * KPA + k][:],
                                         mall[:, :NT])
                    for a in range(1, NA):
                        nc.vector.tensor_mul(tmp[:], G[a * KPA + k][:],
                                             mall[:, a * NT:(a + 1) * NT])
                        nc.vector.tensor_add(sel[:], sel[:], tmp[:])
                    hl = sp.tile([128, NT], BF16, tag=f"hL{k}",
                                 name=f"hL{k}")
                    nc.vector.tensor_mul(
                        hl[:], sel[:],
                        mall[:, (NA + 1) * NT:(NA + 2) * NT])
                    hh = sp.tile([128, NT], BF16, tag=f"hH{k}",
                                 name=f"hH{k}")
                    for n in range(NCLS):
                        dcol = 192 * (n % 2) + 64 * (n // 2)
                        mc = NA * NT + n * 64
                        nc.vector.tensor_mul(hh[:, dcol:dcol + 64],
                                             sel[:, n * 64:(n + 1) * 64],
                                             mall[:, mc:mc + 64])
                    hidL.append(hl)
                    hidH.append(hh)
                return hidL, hidH

            def patch_chunk(ci, mid=None, g2_split=False):
                xa = xs[ci]
                g1s = []
                for h in range(HT):
                    ps = pp.tile([128, 512], F32, tag="ps", name="ps")
                    for d in range(DT):
                        nc.tensor.matmul(
                            ps[:, :CW],
                            w1s[:, h * DT * 128 + d * 128:
                                h * DT * 128 + (d + 1) * 128],
                            xa[:, d * CW:(d + 1) * CW],
                            start=(d == 0), stop=(d == DT - 1))
                    g1 = g1p.tile([128, CW], BF16, tag="g1", name="g1")
                    nc.scalar.activation(g1[:], ps[:, :CW], AF.Gelu,
                                         bias=b1T[:, h:h + 1])
                    g1s.append(g1)
                mid_out = mid() if mid is not None else None
                stg = op.tile([128, DT * CW], BF16, tag="ostg", name="ostg")
                if g2_split:
                    # two h-passes over all dp banks: h16-23's w2 slices
                    # (still in flight at chunk-0 time) are only needed for
                    # the second pass, ~16us later than dp-major order
                    psd = [pp.tile([128, 512], F32, tag="ps", name="ps")
                           for _ in range(DT)]
                    for h0, h1 in ((0, 16), (16, HT)):
                        for dp in range(DT):
                            for h in range(h0, h1):
                                nc.tensor.matmul(
                                    psd[dp][:, :CW],
                                    w2T[:, h * D + dp * 128:
                                        h * D + (dp + 1) * 128],
                                    g1s[h][:, :CW],
                                    start=(h == 0), stop=(h == HT - 1),
                                    skip_group_check=True)
                    for dp in range(DT):
                        nc.vector.tensor_copy(stg[:, dp * CW:(dp + 1) * CW],
                                              psd[dp][:, :CW])
                else:
                    for dp in range(DT):
                        ps = pp.tile([128, 512], F32, tag="ps", name="ps")
                        for h in range(HT):
                            nc.tensor.matmul(
                                ps[:, :CW],
                                w2T[:, h * D + dp * 128:h * D + (dp + 1) * 128],
                                g1s[h][:, :CW],
                                start=(h == 0), stop=(h == HT - 1))
                        nc.vector.tensor_copy(stg[:, dp * CW:(dp + 1) * CW],
                                              ps[:, :CW])
                        if ci == NCH - 1:
                            # tail chunk: per-dp DMAs on the fast queues
                            eng = nc.sync if dp % 2 == 0 else nc.scalar
                            eng.dma_start(
                                pout2_d[ci][:, dp * CW:(dp + 1) * CW],
                                stg[:, dp * CW:(dp + 1) * CW])
                if ci != NCH - 1:
                    nc.gpsimd.dma_start(pout2_d[ci], stg[:])
                return mid_out

            # ---- patch chunk 0, then atom out-GEMM, then chunks 1..3 ----
            hidL, hidH = patch_chunk(0, mid=phase_b, g2_split=True)

            cstg = cp.tile([128, DT * NT], F32, tag="cstg", name="cstg")
            for dp in range(DT):
                psL = pp.tile([128, 512], F32, tag="ps", name="ps")
                n = 0
                for a in range(3):
                    for k in range(KPA):
                        c0 = a * KPA * D + k * D + dp * 128
                        nc.tensor.matmul(
                            psL[:, a * 128:(a + 1) * 128],
                            aoutF[:, c0:c0 + 128],
                            hidL[k][:, a * 128:(a + 1) * 128],
                            start=(n == 0), stop=(n == 3 * KPA - 1))
                        n += 1
                psH = pp.tile([128, 512], F32, tag="ps", name="ps")
                n = 0
                for j in range(2):
                    for k in range(KPA):
                        c0 = (3 + j) * KPA * D + k * D + dp * 128
                        nc.tensor.matmul(
                            psH[:, j * 192:(j + 1) * 192],
                            aoutF[:, c0:c0 + 128],
                            hidH[k][:, j * 192:(j + 1) * 192],
                            start=(n == 0), stop=(n == 2 * KPA - 1))
                        n += 1
                # merge: cstg[slot-major] = psL[slot-major] + psH[parity]
                # (DVE may read only one operand from PSUM: copy, then add)
                nc.vector.tensor_copy(cstg[:, dp * NT:(dp + 1) * NT],
                                      psL[:, :NT])
                for nn in range(NCLS):
                    scol = 192 * (nn % 2) + 64 * (nn // 2)
                    cs = cstg[:, dp * NT + nn * 64:dp * NT + (nn + 1) * 64]
                    nc.vector.tensor_add(cs, cs, psH[:, scol:scol + 64])
            nc.sync.dma_start(cpartT_d[:], cstg[:])

            patch_chunk(1)
            patch_chunk(2)
            patch_chunk(3)

    nc.compile()
    return nc


def _sigmoid(x):
    out = np.empty_like(x)
    pos = x >= 0
    out[pos] = 1.0 / (1.0 + np.exp(-x[pos]))
    ex = np.exp(x[~pos])
    out[~pos] = ex / (1.0 + ex)
    return out


def kernel(x, patch_w1, patch_b1, patch_w2, patch_b2, gate_delta,
           atom_in_w, atom_in_b, atom_out_w, atom_out_b):
    x = np.asarray(x, dtype=np.float32)
    patch_w1 = np.asarray(patch_w1, dtype=np.float32)
    patch_b1 = np.asarray(patch_b1, dtype=np.float32)
    patch_w2 = np.asarray(patch_w2, dtype=np.float32)
    patch_b2 = np.asarray(patch_b2, dtype=np.float32)
    gate_delta = np.asarray(gate_delta, dtype=np.float32)
    atom_in_w = np.asarray(atom_in_w, dtype=np.float32)
    atom_in_b = np.asarray(atom_in_b, dtype=np.float32)
    atom_out_w = np.asarray(atom_out_w, dtype=np.float32)
    atom_out_b = np.asarray(atom_out_b, dtype=np.float32)

    bf = ml_dtypes.bfloat16

    # ---- host routing (tiny) ----
    cls3 = x[:, :NCLS, :]                                   # [B, 6, D]
    logits = np.einsum("bnd,nd->bn", cls3, gate_delta)      # [B, 6] f32
    choose_left = logits >= 0
    p_left = _sigmoid(logits)
    wgt = np.where(choose_left, p_left, 1.0 - p_left).astype(np.float32)
    keys = np.where(choose_left, LEFT_KEYS[None, :], RIGHT_KEYS[None, :])
    dst = (keys % NA).reshape(-1)                           # [B*6]
    wflat = wgt.reshape(-1)

    # token order on device: t = n*64 + b (slot-major)
    src_sm = (keys // NA).T.reshape(-1)                     # [384] slot-major
    left_sm = choose_left.T.reshape(-1)
    w_sm = wgt.T.reshape(-1)

    msrc = (src_sm[None, :] == np.arange(NA)[:, None]).astype(np.float32)
    mL = np.where(left_sm, w_sm, 0.0).astype(np.float32)
    mR = np.where(~left_sm, w_sm, 0.0).astype(np.float32)
    mall_rep = np.ascontiguousarray(np.broadcast_to(
        np.concatenate([msrc.reshape(-1), mL, mR]).reshape(1, (NA + 2) * NT),
        (128, (NA + 2) * NT))).astype(bf)

    # ---- replicated tensors (partition-major packed) ----
    # clsT[p, d*NT + t] = cls_sm[t, d*128+p],  t slot-major (n*64+b)
    cls_sm = np.ascontiguousarray(cls3.transpose(1, 0, 2)).reshape(NT, D)
    clsT = np.ascontiguousarray(
        cls_sm.reshape(NT, DT, 128).transpose(2, 1, 0)
    ).reshape(128, DT * NT).astype(bf)
    # w1s[p, h*768 + d*128 + m] = patch_w1[h*128+m, d*128+p]
    w1s = np.ascontiguousarray(
        patch_w1.reshape(HT, 128, DT, 128).transpose(3, 0, 2, 1)
    ).reshape(128, HT * DT * 128).astype(bf)
    b1T = np.ascontiguousarray(patch_b1.reshape(HT, 128).T)
    # w2T[p, h*D + dp*128 + m] = patch_w2[dp*128+m, h*128+p]
    w2T = np.ascontiguousarray(
        patch_w2.reshape(DT, 128, HT, 128).transpose(3, 2, 0, 1)
    ).reshape(128, HT * D).astype(bf)

    # ---- per-core tensors ----
    patch = x[:, NCLS:, :].reshape(NCORES, TPC, D)
    # xT[p, ci*DT*CW + d*CW + t] = patch[c][ci*CW+t, d*128+p]
    xT_all = np.ascontiguousarray(
        patch.reshape(NCORES, NCH, CW, DT, 128).transpose(0, 4, 1, 3, 2)
    ).reshape(NCORES, 128, NCH * DT * CW).astype(bf)

    ain2_all, ainbT_all, aoutT_all = [], [], []
    for c in range(NCORES):
        hsl = slice(HSH * c, HSH * (c + 1))
        # ain2[a*3+k][p, d*128+m] = atom_in_w[a, hsl0+k*128+m, d*128+p]
        ain2 = np.ascontiguousarray(
            atom_in_w[:, hsl, :].reshape(NA, KPA, 128, DT, 128)
            .transpose(0, 1, 4, 3, 2)).reshape(NG, 128, DT * 128).astype(bf)
        ain2_all.append(ain2)
        ainbT_all.append(np.ascontiguousarray(
            atom_in_b[:, hsl].reshape(NG, 128).T))
        # aoutF[p, a*KPA*D + k*D + dp*128 + m]
        #   = atom_out_w[a, dp*128+m, hsl0+k*128+p]
        aoutF = np.ascontiguousarray(
            atom_out_w[:, :, hsl].reshape(NA, DT, 128, KPA, 128)
            .transpose(4, 0, 3, 1, 2)).reshape(128, NA * KPA * D).astype(bf)
        aoutT_all.append(aoutF)

    in_maps = []
    for c in range(NCORES):
        in_maps.append({
            "xT": xT_all[c], "w1s": w1s, "b1T": b1T, "w2T": w2T,
            "clsT": clsT, "ain2": ain2_all[c], "ainbT": ainbT_all[c],
            "aoutF": aoutT_all[c], "mall": mall_rep,
        })

    nc = _CACHE.get("nc")
    if nc is None:
        nc = _build_program()
        _CACHE["nc"] = nc

    res = run_bass_kernel_spmd(nc, in_maps, core_ids=list(range(NCORES)))
    global LAST_RESULTS
    LAST_RESULTS = res

    # ---- host gather ----
    patch_out = np.empty((B, P, D), dtype=np.float32)
    for c in range(NCORES):
        # pout2[ci][p, d*CW+t] -> [D, TPC]
        po = res.results[c]["pout2"].astype(np.float32)     # [NCH,128,DT*CW]
        po = po.reshape(NCH, 128, DT, CW).transpose(2, 1, 0, 3).reshape(D, TPC)
        patch_out[BPC * c:BPC * (c + 1)] = (
            po.T + patch_b2[None, :]).reshape(BPC, P, D)

    cpart = np.zeros((128, DT * NT), dtype=np.float32)
    for c in range(NCORES):
        cpart += res.results[c]["cpartT"]
    cls_sm_out = cpart.reshape(128, DT, NT).transpose(1, 0, 2).reshape(D, NT).T
    # slot-major [n*64+b] -> [b, n]
    cls_out = np.ascontiguousarray(
        cls_sm_out.reshape(NCLS, B, D).transpose(1, 0, 2))
    cls_out += (wflat[:, None] * atom_out_b[dst, :]).reshape(B, NCLS, D)

    return np.concatenate([cls_out, patch_out], axis=1)


# revision 49
# speedup vs baseline: 1.1668x; 1.0133x over previous
"""Trainium2 Bass kernel for nn_Mlp_moe: dense patch-token MLP + top-1 gated
atom (expert) routing for 6 CLS task tokens.

Sharding over 8 NeuronCores:
  - Patch MLP: data-parallel over batch B=64 -> 8 batches (1568 patch tokens)
    per core. MLP weights replicated (SBUF-resident, bf16).
  - Atom/CLS part: hidden dim H=3072 sharded 8-way (384 per core); every core
    processes all 384 CLS tokens on its H-shard and emits a partial output
    summed on the host. Routing (gate logits/sigmoid/top-1) is computed on
    the host (O(B*6*D), negligible) and shipped as mask vectors.

Structure of the routing: token at slot n either goes left (src=n//2,
dst=3+n%2) or right (src=3+n%2, dst=n//2) with weight w. The in-GEMM is
computed for all 5 atoms (it doubles as DMA-latency cover at kernel start);
the chosen hid is selected with masks; the out-GEMM only computes the two
dst candidates per token (2/5 of the all-atom cost):
  psumL[tok in slots 2a,2a+1] += aout[a]    @ (sel * w*1[right])   a=0,1,2
  psumH[tok, parity j blocks] += aout[3+j]  @ (sel * w*1[left])    j=0,1

DMA: inputs are packed host-side into partition-major slices and issued in
compute-need order, alternating between the two fast HWDGE queues (sync /
scalar); per-(a,k) ain slices and per-4-h-tile w1/w2 slices let compute
start as soon as the first ~1MB lands instead of after whole tensors.
Patch outputs are written in bf16 (host upcasts and adds the bias).
"""

import numpy as np
import ml_dtypes

import concourse.bass as bass
import concourse.bacc as bacc
import concourse.mybir as mybir
from concourse import tile
from concourse.bass_utils import run_bass_kernel_spmd

NCORES = 8
B, NCLS, P, D, H = 64, 6, 196, 768, 3072
NA = 5
HSH = H // NCORES            # 384: per-core atom hidden shard
BPC = B // NCORES            # 8 batches per core
TPC = BPC * P                # 1568 patch tokens per core
NT = B * NCLS                # 384 cls tokens
DT = D // 128                # 6 d-tiles
HT = H // 128                # 24 h-tiles
KPA = HSH // 128             # 3 h-shard tiles per atom
NG = NA * KPA                # 15 (atom, k) in-GEMM groups
CW = 392
NCH = 4
WP = 4                       # h-tiles per w1/w2 DMA piece
NWP = HT // WP               # 6 pieces

LEFT_KEYS = np.array([3, 4, 8, 9, 13, 14], dtype=np.int64)
RIGHT_KEYS = np.array([15, 20, 16, 21, 17, 22], dtype=np.int64)

BF16 = mybir.dt.bfloat16
F32 = mybir.dt.float32
AF = mybir.ActivationFunctionType

_CACHE = {}
LAST_RESULTS = None  # BassKernelResults of the most recent run (for profiling)


def _build_program():
    nc = bacc.Bacc(None, target_bir_lowering=False, debug=False,
                   num_devices=NCORES)

    # ---- DRAM inputs (partition-major packed, see host layouts below) ----
    clsT_d = nc.dram_tensor("clsT", [128, DT * NT], BF16,
                            kind="ExternalInput")
    ainbT_d = nc.dram_tensor("ainbT", [128, NG], F32, kind="ExternalInput")
    b1T_d = nc.dram_tensor("b1T", [128, HT], F32, kind="ExternalInput")
    ain2_d = nc.dram_tensor("ain2", [NG, 128, DT * 128], BF16,
                            kind="ExternalInput")
    xT_d = nc.dram_tensor("xT", [128, NCH * DT * CW], BF16,
                          kind="ExternalInput")
    w1s_d = nc.dram_tensor("w1s", [128, HT * DT * 128], BF16,
                           kind="ExternalInput")
    w2T_d = nc.dram_tensor("w2T", [128, HT * D], BF16, kind="ExternalInput")
    mall_d = nc.dram_tensor("mall", [128, (NA + 2) * NT], BF16,
                            kind="ExternalInput")
    aoutF_d = nc.dram_tensor("aoutF", [128, NA * KPA * D], BF16,
                             kind="ExternalInput")
    # outputs: pout2[ci][p, d*CW + t] = patch_out_pre_bias[d*128+p, ci*CW+t]
    pout2_d = nc.dram_tensor("pout2", [NCH, 128, DT * CW], BF16,
                             kind="ExternalOutput")
    cpartT_d = nc.dram_tensor("cpartT", [128, DT * NT], F32,
                              kind="ExternalOutput")

    with tile.TileContext(nc) as tc:
        with (
            tc.tile_pool(name="w", bufs=1) as wp,
            tc.tile_pool(name="gat", bufs=1) as gp,
            tc.tile_pool(name="sel", bufs=1) as sp,
            tc.tile_pool(name="g1", bufs=24) as g1p,
            tc.tile_pool(name="ostg", bufs=1) as op,
            tc.tile_pool(name="cstg", bufs=1) as cp,
            tc.tile_pool(name="ps", bufs=8, space="PSUM") as pp,
        ):
            # ---- resident tiles ----
            clsT = wp.tile([128, DT * NT], BF16, tag="cls", name="cls")
            ainbT = wp.tile([128, NG], F32, tag="ainb", name="ainb")
            b1T = wp.tile([128, HT], F32, tag="b1", name="b1")
            ain2 = [wp.tile([128, DT * 128], BF16, tag=f"ain{g}",
                            name=f"ain{g}") for g in range(NG)]
            xs = [wp.tile([128, DT * CW], BF16, tag=f"x{c}", name=f"x{c}")
                  for c in range(NCH)]
            w1s = wp.tile([128, HT * DT * 128], BF16, tag="w1", name="w1")
            w2T = wp.tile([128, HT * D], BF16, tag="w2", name="w2")
            # masks combined into one tile: cols [0, NA*NT) = msrc per atom,
            # [NA*NT, (NA+1)*NT) = mL, [(NA+1)*NT, (NA+2)*NT) = mR
            mall = wp.tile([128, (NA + 2) * NT], BF16, tag="mall",
                           name="mall")
            aoutF = wp.tile([128, NA * KPA * D], BF16, tag="aoF",
                            name="aoF")

            # ---- DMA emission machinery ----
            # dma_start issues each wait on one of 8 round-robin completion
            # semaphore lanes (issue N blocks on completion of issue N-8), so
            # a long run of back-to-back dma_starts serializes the issuing
            # engine's queue.  Only the startup-critical transfers are issued
            # up front; the rest are threaded into the compute stream at
            # points where their lane-waits have long resolved.  Alternating
            # sync/scalar (the two HWDGE rings) keeps both rings pulling, and
            # FIFO-per-ring makes data land in need order.
            XW = DT * CW
            W1P = WP * DT * 128
            W2P = WP * D
            items = {}
            for j in range(3):  # cls in d-pair thirds
                c0, c1 = j * 2 * NT, (j + 1) * 2 * NT
                items[f"cls{j}"] = (clsT[:, c0:c1], clsT_d[:, c0:c1])
            items["ainb"] = (ainbT[:], ainbT_d[:])
            items["b1"] = (b1T[:], b1T_d[:])
            for g in range(NG):
                items[f"ain{g}"] = (ain2[g][:], ain2_d[g])
            for hf in range(2):
                c0 = hf * (XW // 2)
                items[f"x0_{hf}"] = (xs[0][:, c0:c0 + XW // 2],
                                     xT_d[:, c0:c0 + XW // 2])
            for c in range(1, NCH):
                items[f"x{c}"] = (xs[c][:], xT_d[:, c * XW:(c + 1) * XW])
            for j in range(NWP):
                items[f"w1p{j}"] = (w1s[:, j * W1P:(j + 1) * W1P],
                                    w1s_d[:, j * W1P:(j + 1) * W1P])
            for j in range(2):  # w2 in halves
                c0, c1 = j * 12 * D, (j + 1) * 12 * D
                items[f"w2p{j}"] = (w2T[:, c0:c1], w2T_d[:, c0:c1])
            items["mall"] = (mall[:], mall_d[:])
            AOH = NA * KPA * D // 2
            for j in range(2):  # aout in halves
                c0, c1 = j * AOH, (j + 1) * AOH if j else AOH
                items[f"aoF{j}"] = (aoutF[:, c0:c1], aoutF_d[:, c0:c1])

            dma_i = [0]

            def dma(*names, eng=None):
                for nm in names:
                    dst, src = items.pop(nm)
                    e = eng
                    if e is None:
                        e = nc.sync if dma_i[0] % 2 == 0 else nc.scalar
                        dma_i[0] += 1
                    e.dma_start(dst, src)

            # scalar (which also runs the gelus) gets ONLY the small
            # startup-critical transfers; its dma-issue chain ends by ~16us.
            # sync gets everything else up front in need order: its
            # semaphore-lane waits serialize harmlessly (no compute there),
            # and a lone HWDGE ring saturates HBM (~350GB/s measured).
            # x1..x3 ride at the end of the sync chain: the lane
            # serialization delays their issue past the startup-critical
            # window, so they never compete with cls/ain/w1 for HBM.
            dma("cls1", "ainb", "b1",
                *[f"ain{g}" for g in range(0, NG, 2)],
                eng=nc.scalar)
            dma("cls0", "cls2",
                *[f"ain{g}" for g in range(1, NG, 2)],
                "x0_0", "x0_1", *[f"w1p{j}" for j in range(NWP)],
                "w2p0", "w2p1", "mall", "aoF0", "aoF1",
                "x1", "x2", "x3",
                eng=nc.sync)

            # ---- PE warm-up: dummy matmuls while the first DMAs land, so
            # the HAM clock-gate reaches 8/8 before real work starts ----
            warm = wp.tile([128, 512], BF16, tag="warm", name="warm")
            nc.vector.memset(warm[:], 0)
            wps = pp.tile([128, 512], F32, tag="ps", name="ps")
            # the HAM activity window is free-running, so >=2 windows (6.8us)
            # of PE busy guarantees the warm flip before real work starts
            NWARM = 12
            for i in range(NWARM):
                nc.tensor.matmul(wps[:, :512], warm[:, :128], warm[:],
                                 start=(i == 0), stop=(i == NWARM - 1))

            # ---- phase A: atom in-GEMM + gelu, all 5 atoms, (a,k)-major ---
            G = {}
            for g in range(NG):
                ps = pp.tile([128, 512], F32, tag="ps", name="ps")
                for d in range(DT):
                    nc.tensor.matmul(ps[:, :NT],
                                     ain2[g][:, d * 128:(d + 1) * 128],
                                     clsT[:, d * NT:(d + 1) * NT],
                                     start=(d == 0), stop=(d == DT - 1))
                gt = gp.tile([128, NT], BF16, tag=f"g{g}", name=f"g{g}")
                nc.scalar.activation(gt[:], ps[:, :NT], AF.Gelu,
                                     bias=ainbT[:, g:g + 1])
                G[g] = gt

            def phase_b():
                # ---- phase B (DVE): src-select, then dst-candidate masks
                hidL, hidH = [], []
                for k in range(KPA):
                    sel = sp.tile([128, NT], BF16, tag=f"sel{k}",
                                  name=f"sel{k}")
                    tmp = sp.tile([128, NT], BF16, tag="tmp", name="tmp")
                    nc.vector.tensor_mul(sel[:], G[0 * KPA + k][:],
                                         mall[:, :NT])
                    for a in range(1, NA):
                        nc.vector.tensor_mul(tmp[:], G[a * KPA + k][:],
                                             mall[:, a * NT:(a + 1) * NT])
                        nc.vector.tensor_add(sel[:], sel[:], tmp[:])
                    hl = sp.tile([128, NT], BF16, tag=f"hL{k}",
                                 name=f"hL{k}")
                    nc.vector.tensor_mul(
                        hl[:], sel[:],
                        mall[:, (NA + 1) * NT:(NA + 2) * NT])
                    hh = sp.tile([128, NT], BF16, tag=f"hH{k}",
                                 name=f"hH{k}")
                    for n in range(NCLS):
                        dcol = 192 * (n % 2) + 64 * (n // 2)
                        mc = NA * NT + n * 64
                        nc.vector.tensor_mul(hh[:, dcol:dcol + 64],
                                             sel[:, n * 64:(n + 1) * 64],
                                             mall[:, mc:mc + 64])
                    hidL.append(hl)
                    hidH.append(hh)
                return hidL, hidH

            def patch_chunk(ci, mid=None, g2_split=False):
                xa = xs[ci]
                g1s = []
                for h in range(HT):
                    ps = pp.tile([128, 512], F32, tag="ps", name="ps")
                    for d in range(DT):
                        nc.tensor.matmul(
                            ps[:, :CW],
                            w1s[:, h * DT * 128 + d * 128:
                                h * DT * 128 + (d + 1) * 128],
                            xa[:, d * CW:(d + 1) * CW],
                            start=(d == 0), stop=(d == DT - 1))
                    g1 = g1p.tile([128, CW], BF16, tag="g1", name="g1")
                    nc.scalar.activation(g1[:], ps[:, :CW], AF.Gelu,
                                         bias=b1T[:, h:h + 1])
                    g1s.append(g1)
                mid_out = mid() if mid is not None else None
                stg = op.tile([128, DT * CW], BF16, tag="ostg", name="ostg")
                if g2_split:
                    # two h-passes over all dp banks: h16-23's w2 slices
                    # (still in flight at chunk-0 time) are only needed for
                    # the second pass, ~16us later than dp-major order
                    psd = [pp.tile([128, 512], F32, tag="ps", name="ps")
                           for _ in range(DT)]
                    for h0, h1 in ((0, 16), (16, HT)):
                        for dp in range(DT):
                            for h in range(h0, h1):
                                nc.tensor.matmul(
                                    psd[dp][:, :CW],
                                    w2T[:, h * D + dp * 128:
                                        h * D + (dp + 1) * 128],
                                    g1s[h][:, :CW],
                                    start=(h == 0), stop=(h == HT - 1),
                                    skip_group_check=True)
                    for dp in range(DT):
                        nc.vector.tensor_copy(stg[:, dp * CW:(dp + 1) * CW],
                                              psd[dp][:, :CW])
                else:
                    for dp in range(DT):
                        ps = pp.tile([128, 512], F32, tag="ps", name="ps")
                        for h in range(HT):
                            nc.tensor.matmul(
                                ps[:, :CW],
                                w2T[:, h * D + dp * 128:h * D + (dp + 1) * 128],
                                g1s[h][:, :CW],
                                start=(h == 0), stop=(h == HT - 1))
                        nc.vector.tensor_copy(stg[:, dp * CW:(dp + 1) * CW],
                                              ps[:, :CW])
                        if ci == NCH - 1:
                            # tail chunk: per-dp DMAs on the fast queues
                            eng = nc.sync if dp % 2 == 0 else nc.scalar
                            eng.dma_start(
                                pout2_d[ci][:, dp * CW:(dp + 1) * CW],
                                stg[:, dp * CW:(dp + 1) * CW])
                if ci != NCH - 1:
                    nc.gpsimd.dma_start(pout2_d[ci], stg[:])
                return mid_out

            # ---- patch chunk 0, then atom out-GEMM, then chunks 1..3 ----
            hidL, hidH = patch_chunk(0, mid=phase_b, g2_split=True)

            cstg = cp.tile([128, DT * NT], F32, tag="cstg", name="cstg")
            for dp in range(DT):
                psL = pp.tile([128, 512], F32, tag="ps", name="ps")
                n = 0
                for a in range(3):
                    for k in range(KPA):
                        c0 = a * KPA * D + k * D + dp * 128
                        nc.tensor.matmul(
                            psL[:, a * 128:(a + 1) * 128],
                            aoutF[:, c0:c0 + 128],
                            hidL[k][:, a * 128:(a + 1) * 128],
                            start=(n == 0), stop=(n == 3 * KPA - 1))
                        n += 1
                psH = pp.tile([128, 512], F32, tag="ps", name="ps")
                n = 0
                for j in range(2):
                    for k in range(KPA):
                        c0 = (3 + j) * KPA * D + k * D + dp * 128
                        nc.tensor.matmul(
                            psH[:, j * 192:(j + 1) * 192],
                            aoutF[:, c0:c0 + 128],
                            hidH[k][:, j * 192:(j + 1) * 192],
                            start=(n == 0), stop=(n == 2 * KPA - 1))
                        n += 1
                # merge: cstg[slot-major] = psL[slot-major] + psH[parity]
                # (DVE may read only one operand from PSUM: copy, then add)
                nc.vector.tensor_copy(cstg[:, dp * NT:(dp + 1) * NT],
                                      psL[:, :NT])
                for nn in range(NCLS):
                    scol = 192 * (nn % 2) + 64 * (nn // 2)
                    cs = cstg[:, dp * NT + nn * 64:dp * NT + (nn + 1) * 64]
                    nc.vector.tensor_add(cs, cs, psH[:, scol:scol + 64])
            nc.sync.dma_start(cpartT_d[:], cstg[:])

            patch_chunk(1)
            patch_chunk(2)
            patch_chunk(3)

    nc.compile()
    return nc


def _sigmoid(x):
    out = np.empty_like(x)
    pos = x >= 0
    out[pos] = 1.0 / (1.0 + np.exp(-x[pos]))
    ex = np.exp(x[~pos])
    out[~pos] = ex / (1.0 + ex)
    return out


def kernel(x, patch_w1, patch_b1, patch_w2, patch_b2, gate_delta,
           atom_in_w, atom_in_b, atom_out_w, atom_out_b):
    x = np.asarray(x, dtype=np.float32)
    patch_w1 = np.asarray(patch_w1, dtype=np.float32)
    patch_b1 = np.asarray(patch_b1, dtype=np.float32)
    patch_w2 = np.asarray(patch_w2, dtype=np.float32)
    patch_b2 = np.asarray(patch_b2, dtype=np.float32)
    gate_delta = np.asarray(gate_delta, dtype=np.float32)
    atom_in_w = np.asarray(atom_in_w, dtype=np.float32)
    atom_in_b = np.asarray(atom_in_b, dtype=np.float32)
    atom_out_w = np.asarray(atom_out_w, dtype=np.float32)
    atom_out_b = np.asarray(atom_out_b, dtype=np.float32)

    bf = ml_dtypes.bfloat16

    # ---- host routing (tiny) ----
    cls3 = x[:, :NCLS, :]                                   # [B, 6, D]
    logits = np.einsum("bnd,nd->bn", cls3, gate_delta)      # [B, 6] f32
    choose_left = logits >= 0
    p_left = _sigmoid(logits)
    wgt = np.where(choose_left, p_left, 1.0 - p_left).astype(np.float32)
    keys = np.where(choose_left, LEFT_KEYS[None, :], RIGHT_KEYS[None, :])
    dst = (keys % NA).reshape(-1)                           # [B*6]
    wflat = wgt.reshape(-1)

    # token order on device: t = n*64 + b (slot-major)
    src_sm = (keys // NA).T.reshape(-1)                     # [384] slot-major
    left_sm = choose_left.T.reshape(-1)
    w_sm = wgt.T.reshape(-1)

    msrc = (src_sm[None, :] == np.arange(NA)[:, None]).astype(np.float32)
    mL = np.where(left_sm, w_sm, 0.0).astype(np.float32)
    mR = np.where(~left_sm, w_sm, 0.0).astype(np.float32)
    mall_rep = np.ascontiguousarray(np.broadcast_to(
        np.concatenate([msrc.reshape(-1), mL, mR]).reshape(1, (NA + 2) * NT),
        (128, (NA + 2) * NT))).astype(bf)

    # ---- replicated tensors (partition-major packed) ----
    # clsT[p, d*NT + t] = cls_sm[t, d*128+p],  t slot-major (n*64+b)
    cls_sm = np.ascontiguousarray(cls3.transpose(1, 0, 2)).reshape(NT, D)
    clsT = np.ascontiguousarray(
        cls_sm.reshape(NT, DT, 128).transpose(2, 1, 0)
    ).reshape(128, DT * NT).astype(bf)
    # w1s[p, h*768 + d*128 + m] = patch_w1[h*128+m, d*128+p]
    w1s = np.ascontiguousarray(
        patch_w1.reshape(HT, 128, DT, 128).transpose(3, 0, 2, 1)
    ).reshape(128, HT * DT * 128).astype(bf)
    b1T = np.ascontiguousarray(patch_b1.reshape(HT, 128).T)
    # w2T[p, h*D + dp*128 + m] = patch_w2[dp*128+m, h*128+p]
    w2T = np.ascontiguousarray(
        patch_w2.reshape(DT, 128, HT, 128).transpose(3, 2, 0, 1)
    ).reshape(128, HT * D).astype(bf)

    # ---- per-core tensors ----
    patch = x[:, NCLS:, :].reshape(NCORES, TPC, D)
    # xT[p, ci*DT*CW + d*CW + t] = patch[c][ci*CW+t, d*128+p]
    xT_all = np.ascontiguousarray(
        patch.reshape(NCORES, NCH, CW, DT, 128).transpose(0, 4, 1, 3, 2)
    ).reshape(NCORES, 128, NCH * DT * CW).astype(bf)

    ain2_all, ainbT_all, aoutT_all = [], [], []
    for c in range(NCORES):
        hsl = slice(HSH * c, HSH * (c + 1))
        # ain2[a*3+k][p, d*128+m] = atom_in_w[a, hsl0+k*128+m, d*128+p]
        ain2 = np.ascontiguousarray(
            atom_in_w[:, hsl, :].reshape(NA, KPA, 128, DT, 128)
            .transpose(0, 1, 4, 3, 2)).reshape(NG, 128, DT * 128).astype(bf)
        ain2_all.append(ain2)
        ainbT_all.append(np.ascontiguousarray(
            atom_in_b[:, hsl].reshape(NG, 128).T))
        # aoutF[p, a*KPA*D + k*D + dp*128 + m]
        #   = atom_out_w[a, dp*128+m, hsl0+k*128+p]
        aoutF = np.ascontiguousarray(
            atom_out_w[:, :, hsl].reshape(NA, DT, 128, KPA, 128)
            .transpose(4, 0, 3, 1, 2)).reshape(128, NA * KPA * D).astype(bf)
        aoutT_all.append(aoutF)

    in_maps = []
    for c in range(NCORES):
        in_maps.append({
            "xT": xT_all[c], "w1s": w1s, "b1T": b1T, "w2T": w2T,
            "clsT": clsT, "ain2": ain2_all[c], "ainbT": ainbT_all[c],
            "aoutF": aoutT_all[c], "mall": mall_rep,
        })

    nc = _CACHE.get("nc")
    if nc is None:
        nc = _build_program()
        _CACHE["nc"] = nc

    res = run_bass_kernel_spmd(nc, in_maps, core_ids=list(range(NCORES)))
    global LAST_RESULTS
    LAST_RESULTS = res

    # ---- host gather ----
    patch_out = np.empty((B, P, D), dtype=np.float32)
    for c in range(NCORES):
        # pout2[ci][p, d*CW+t] -> [D, TPC]
        po = res.results[c]["pout2"].astype(np.float32)     # [NCH,128,DT*CW]
        po = po.reshape(NCH, 128, DT, CW).transpose(2, 1, 0, 3).reshape(D, TPC)
        patch_out[BPC * c:BPC * (c + 1)] = (
            po.T + patch_b2[None, :]).reshape(BPC, P, D)

    cpart = np.zeros((128, DT * NT), dtype=np.float32)
    for c in range(NCORES):
        cpart += res.results[c]["cpartT"]
    cls_sm_out = cpart.reshape(128, DT, NT).transpose(1, 0, 2).reshape(D, NT).T
    # slot-major [n*64+b] -> [b, n]
    cls_out = np.ascontiguousarray(
        cls_sm_out.reshape(NCLS, B, D).transpose(1, 0, 2))
    cls_out += (wflat[:, None] * atom_out_b[dst, :]).reshape(B, NCLS, D)

    return np.concatenate([cls_out, patch_out], axis=1)


# revision 51
# speedup vs baseline: 1.1731x; 1.0055x over previous
"""Trainium2 Bass kernel for nn_Mlp_moe: dense patch-token MLP + top-1 gated
atom (expert) routing for 6 CLS task tokens.

Sharding over 8 NeuronCores:
  - Patch MLP: data-parallel over batch B=64 -> 8 batches (1568 patch tokens)
    per core. MLP weights replicated (SBUF-resident, bf16).
  - Atom/CLS part: hidden dim H=3072 sharded 8-way (384 per core); every core
    processes all 384 CLS tokens on its H-shard and emits a partial output
    summed on the host. Routing (gate logits/sigmoid/top-1) is computed on
    the host (O(B*6*D), negligible) and shipped as mask vectors.

Structure of the routing: token at slot n either goes left (src=n//2,
dst=3+n%2) or right (src=3+n%2, dst=n//2) with weight w. The in-GEMM is
computed for all 5 atoms (it doubles as DMA-latency cover at kernel start);
the chosen hid is selected with masks; the out-GEMM only computes the two
dst candidates per token (2/5 of the all-atom cost):
  psumL[tok in slots 2a,2a+1] += aout[a]    @ (sel * w*1[right])   a=0,1,2
  psumH[tok, parity j blocks] += aout[3+j]  @ (sel * w*1[left])    j=0,1

DMA: inputs are packed host-side into partition-major slices and issued in
compute-need order, alternating between the two fast HWDGE queues (sync /
scalar); per-(a,k) ain slices and per-4-h-tile w1/w2 slices let compute
start as soon as the first ~1MB lands instead of after whole tensors.
Patch outputs are written in bf16 (host upcasts and adds the bias).
"""

import numpy as np
import ml_dtypes

import concourse.bass as bass
import concourse.bacc as bacc
import concourse.mybir as mybir
from concourse import tile
from concourse.bass_utils import run_bass_kernel_spmd

NCORES = 8
B, NCLS, P, D, H = 64, 6, 196, 768, 3072
NA = 5
HSH = H // NCORES            # 384: per-core atom hidden shard
BPC = B // NCORES            # 8 batches per core
TPC = BPC * P                # 1568 patch tokens per core
NT = B * NCLS                # 384 cls tokens
DT = D // 128                # 6 d-tiles
HT = H // 128                # 24 h-tiles
KPA = HSH // 128             # 3 h-shard tiles per atom
NG = NA * KPA                # 15 (atom, k) in-GEMM groups
CW = 392
NCH = 4
WP = 4                       # h-tiles per w1/w2 DMA piece
NWP = HT // WP               # 6 pieces

LEFT_KEYS = np.array([3, 4, 8, 9, 13, 14], dtype=np.int64)
RIGHT_KEYS = np.array([15, 20, 16, 21, 17, 22], dtype=np.int64)

BF16 = mybir.dt.bfloat16
F32 = mybir.dt.float32
AF = mybir.ActivationFunctionType

_CACHE = {}
LAST_RESULTS = None  # BassKernelResults of the most recent run (for profiling)


def _build_program():
    nc = bacc.Bacc(None, target_bir_lowering=False, debug=False,
                   num_devices=NCORES)

    # ---- DRAM inputs (partition-major packed, see host layouts below) ----
    clsT_d = nc.dram_tensor("clsT", [128, DT * NT], BF16,
                            kind="ExternalInput")
    ainbT_d = nc.dram_tensor("ainbT", [128, NG], F32, kind="ExternalInput")
    b1T_d = nc.dram_tensor("b1T", [128, HT], F32, kind="ExternalInput")
    ain2_d = nc.dram_tensor("ain2", [NG, 128, DT * 128], BF16,
                            kind="ExternalInput")
    xT_d = nc.dram_tensor("xT", [128, NCH * DT * CW], BF16,
                          kind="ExternalInput")
    w1s_d = nc.dram_tensor("w1s", [128, HT * DT * 128], BF16,
                           kind="ExternalInput")
    w2T_d = nc.dram_tensor("w2T", [128, HT * D], BF16, kind="ExternalInput")
    mall_d = nc.dram_tensor("mall", [128, (NA + 2) * NT], BF16,
                            kind="ExternalInput")
    aoutF_d = nc.dram_tensor("aoutF", [128, NA * KPA * D], BF16,
                             kind="ExternalInput")
    # outputs: pout2[ci][p, d*CW + t] = patch_out_pre_bias[d*128+p, ci*CW+t]
    pout2_d = nc.dram_tensor("pout2", [NCH, 128, DT * CW], BF16,
                             kind="ExternalOutput")
    cpartT_d = nc.dram_tensor("cpartT", [128, DT * NT], F32,
                              kind="ExternalOutput")

    with tile.TileContext(nc) as tc:
        with (
            tc.tile_pool(name="w", bufs=1) as wp,
            tc.tile_pool(name="gat", bufs=1) as gp,
            tc.tile_pool(name="sel", bufs=1) as sp,
            tc.tile_pool(name="g1", bufs=24) as g1p,
            tc.tile_pool(name="ostg", bufs=1) as op,
            tc.tile_pool(name="cstg", bufs=1) as cp,
            tc.tile_pool(name="ps", bufs=8, space="PSUM") as pp,
        ):
            # ---- resident tiles ----
            clsT = wp.tile([128, DT * NT], BF16, tag="cls", name="cls")
            ainbT = wp.tile([128, NG], F32, tag="ainb", name="ainb")
            b1T = wp.tile([128, HT], F32, tag="b1", name="b1")
            ain2 = [wp.tile([128, DT * 128], BF16, tag=f"ain{g}",
                            name=f"ain{g}") for g in range(NG)]
            xs = [wp.tile([128, DT * CW], BF16, tag=f"x{c}", name=f"x{c}")
                  for c in range(NCH)]
            w1s = wp.tile([128, HT * DT * 128], BF16, tag="w1", name="w1")
            w2T = wp.tile([128, HT * D], BF16, tag="w2", name="w2")
            # masks combined into one tile: cols [0, NA*NT) = msrc per atom,
            # [NA*NT, (NA+1)*NT) = mL, [(NA+1)*NT, (NA+2)*NT) = mR
            mall = wp.tile([128, (NA + 2) * NT], BF16, tag="mall",
                           name="mall")
            aoutF = wp.tile([128, NA * KPA * D], BF16, tag="aoF",
                            name="aoF")

            # ---- DMA emission machinery ----
            # dma_start issues each wait on one of 8 round-robin completion
            # semaphore lanes (issue N blocks on completion of issue N-8), so
            # a long run of back-to-back dma_starts serializes the issuing
            # engine's queue.  Only the startup-critical transfers are issued
            # up front; the rest are threaded into the compute stream at
            # points where their lane-waits have long resolved.  Alternating
            # sync/scalar (the two HWDGE rings) keeps both rings pulling, and
            # FIFO-per-ring makes data land in need order.
            XW = DT * CW
            W1P = WP * DT * 128
            W2P = WP * D
            items = {}
            for j in range(3):  # cls in d-pair thirds
                c0, c1 = j * 2 * NT, (j + 1) * 2 * NT
                items[f"cls{j}"] = (clsT[:, c0:c1], clsT_d[:, c0:c1])
            items["ainb"] = (ainbT[:], ainbT_d[:])
            items["b1"] = (b1T[:], b1T_d[:])
            for g in range(NG):
                items[f"ain{g}"] = (ain2[g][:], ain2_d[g])
            for hf in range(2):
                c0 = hf * (XW // 2)
                items[f"x0_{hf}"] = (xs[0][:, c0:c0 + XW // 2],
                                     xT_d[:, c0:c0 + XW // 2])
            for c in range(1, NCH):
                items[f"x{c}"] = (xs[c][:], xT_d[:, c * XW:(c + 1) * XW])
            for j in range(NWP):
                items[f"w1p{j}"] = (w1s[:, j * W1P:(j + 1) * W1P],
                                    w1s_d[:, j * W1P:(j + 1) * W1P])
            for j in range(2):  # w2 in halves
                c0, c1 = j * 12 * D, (j + 1) * 12 * D
                items[f"w2p{j}"] = (w2T[:, c0:c1], w2T_d[:, c0:c1])
            items["mall"] = (mall[:], mall_d[:])
            AOH = NA * KPA * D // 2
            for j in range(2):  # aout in halves
                c0, c1 = j * AOH, (j + 1) * AOH if j else AOH
                items[f"aoF{j}"] = (aoutF[:, c0:c1], aoutF_d[:, c0:c1])

            dma_i = [0]

            def dma(*names, eng=None):
                for nm in names:
                    dst, src = items.pop(nm)
                    e = eng
                    if e is None:
                        e = nc.sync if dma_i[0] % 2 == 0 else nc.scalar
                        dma_i[0] += 1
                    e.dma_start(dst, src)

            # scalar (which also runs the gelus) gets ONLY the small
            # startup-critical transfers; its dma-issue chain ends by ~16us.
            # sync gets everything else up front in need order: its
            # semaphore-lane waits serialize harmlessly (no compute there),
            # and a lone HWDGE ring saturates HBM (~350GB/s measured).
            # x1..x3 ride at the end of the sync chain: the lane
            # serialization delays their issue past the startup-critical
            # window, so they never compete with cls/ain/w1 for HBM.
            dma("cls1", *[f"ain{g}" for g in range(0, NG, 2)],
                eng=nc.scalar)
            dma("cls0", "cls2", "ainb", "b1",
                *[f"ain{g}" for g in range(1, NG, 2)],
                "x0_0", "x0_1", *[f"w1p{j}" for j in range(NWP)],
                "w2p0", "w2p1", "mall", "aoF0", "aoF1",
                "x1", "x2", "x3",
                eng=nc.sync)

            # ---- PE warm-up: dummy matmuls while the first DMAs land, so
            # the HAM clock-gate reaches 8/8 before real work starts ----
            warm = wp.tile([128, 512], BF16, tag="warm", name="warm")
            nc.vector.memset(warm[:], 0)
            wps = pp.tile([128, 512], F32, tag="ps", name="ps")
            # the HAM activity window is free-running, so ~2 windows (6.8us)
            # of PE busy guarantees the warm flip before real work starts
            NWARM = 12
            for i in range(NWARM):
                nc.tensor.matmul(wps[:, :512], warm[:, :128], warm[:],
                                 start=(i == 0), stop=(i == NWARM - 1))

            # ---- phase A: atom in-GEMM + gelu, all 5 atoms, (a,k)-major ---
            G = {}
            for g in range(NG):
                ps = pp.tile([128, 512], F32, tag="ps", name="ps")
                for d in range(DT):
                    nc.tensor.matmul(ps[:, :NT],
                                     ain2[g][:, d * 128:(d + 1) * 128],
                                     clsT[:, d * NT:(d + 1) * NT],
                                     start=(d == 0), stop=(d == DT - 1))
                gt = gp.tile([128, NT], BF16, tag=f"g{g}", name=f"g{g}")
                nc.scalar.activation(gt[:], ps[:, :NT], AF.Gelu,
                                     bias=ainbT[:, g:g + 1])
                G[g] = gt

            def phase_b():
                # ---- phase B (DVE): src-select, then dst-candidate masks
                hidL, hidH = [], []
                for k in range(KPA):
                    sel = sp.tile([128, NT], BF16, tag=f"sel{k}",
                                  name=f"sel{k}")
                    tmp = sp.tile([128, NT], BF16, tag="tmp", name="tmp")
                    nc.vector.tensor_mul(sel[:], G[0 * KPA + k][:],
                                         mall[:, :NT])
                    for a in range(1, NA):
                        nc.vector.tensor_mul(tmp[:], G[a * KPA + k][:],
                                             mall[:, a * NT:(a + 1) * NT])
                        nc.vector.tensor_add(sel[:], sel[:], tmp[:])
                    hl = sp.tile([128, NT], BF16, tag=f"hL{k}",
                                 name=f"hL{k}")
                    nc.vector.tensor_mul(
                        hl[:], sel[:],
                        mall[:, (NA + 1) * NT:(NA + 2) * NT])
                    hh = sp.tile([128, NT], BF16, tag=f"hH{k}",
                                 name=f"hH{k}")
                    for n in range(NCLS):
                        dcol = 192 * (n % 2) + 64 * (n // 2)
                        mc = NA * NT + n * 64
                        nc.vector.tensor_mul(hh[:, dcol:dcol + 64],
                                             sel[:, n * 64:(n + 1) * 64],
                                             mall[:, mc:mc + 64])
                    hidL.append(hl)
                    hidH.append(hh)
                return hidL, hidH

            def patch_chunk(ci, mid=None, g2_split=False):
                xa = xs[ci]
                g1s = []
                for h in range(HT):
                    ps = pp.tile([128, 512], F32, tag="ps", name="ps")
                    for d in range(DT):
                        nc.tensor.matmul(
                            ps[:, :CW],
                            w1s[:, h * DT * 128 + d * 128:
                                h * DT * 128 + (d + 1) * 128],
                            xa[:, d * CW:(d + 1) * CW],
                            start=(d == 0), stop=(d == DT - 1))
                    g1 = g1p.tile([128, CW], BF16, tag="g1", name="g1")
                    nc.scalar.activation(g1[:], ps[:, :CW], AF.Gelu,
                                         bias=b1T[:, h:h + 1])
                    g1s.append(g1)
                mid_out = mid() if mid is not None else None
                stg = op.tile([128, DT * CW], BF16, tag="ostg", name="ostg")
                if g2_split:
                    # two h-passes over all dp banks: h16-23's w2 slices
                    # (still in flight at chunk-0 time) are only needed for
                    # the second pass, ~16us later than dp-major order
                    psd = [pp.tile([128, 512], F32, tag="ps", name="ps")
                           for _ in range(DT)]
                    for h0, h1 in ((0, 16), (16, HT)):
                        for dp in range(DT):
                            for h in range(h0, h1):
                                nc.tensor.matmul(
                                    psd[dp][:, :CW],
                                    w2T[:, h * D + dp * 128:
                                        h * D + (dp + 1) * 128],
                                    g1s[h][:, :CW],
                                    start=(h == 0), stop=(h == HT - 1),
                                    skip_group_check=True)
                    for dp in range(DT):
                        nc.vector.tensor_copy(stg[:, dp * CW:(dp + 1) * CW],
                                              psd[dp][:, :CW])
                else:
                    for dp in range(DT):
                        ps = pp.tile([128, 512], F32, tag="ps", name="ps")
                        for h in range(HT):
                            nc.tensor.matmul(
                                ps[:, :CW],
                                w2T[:, h * D + dp * 128:h * D + (dp + 1) * 128],
                                g1s[h][:, :CW],
                                start=(h == 0), stop=(h == HT - 1))
                        nc.vector.tensor_copy(stg[:, dp * CW:(dp + 1) * CW],
                                              ps[:, :CW])
                        if ci == NCH - 1:
                            # tail chunk: per-dp DMAs on the fast queues
                            eng = nc.sync if dp % 2 == 0 else nc.scalar
                            eng.dma_start(
                                pout2_d[ci][:, dp * CW:(dp + 1) * CW],
                                stg[:, dp * CW:(dp + 1) * CW])
                if ci != NCH - 1:
                    nc.gpsimd.dma_start(pout2_d[ci], stg[:])
                return mid_out

            # ---- patch chunk 0, then atom out-GEMM, then chunks 1..3 ----
            hidL, hidH = patch_chunk(0, mid=phase_b, g2_split=True)

            cstg = cp.tile([128, DT * NT], F32, tag="cstg", name="cstg")
            for dp in range(DT):
                psL = pp.tile([128, 512], F32, tag="ps", name="ps")
                n = 0
                for a in range(3):
                    for k in range(KPA):
                        c0 = a * KPA * D + k * D + dp * 128
                        nc.tensor.matmul(
                            psL[:, a * 128:(a + 1) * 128],
                            aoutF[:, c0:c0 + 128],
                            hidL[k][:, a * 128:(a + 1) * 128],
                            start=(n == 0), stop=(n == 3 * KPA - 1))
                        n += 1
                psH = pp.tile([128, 512], F32, tag="ps", name="ps")
                n = 0
                for j in range(2):
                    for k in range(KPA):
                        c0 = (3 + j) * KPA * D + k * D + dp * 128
                        nc.tensor.matmul(
                            psH[:, j * 192:(j + 1) * 192],
                            aoutF[:, c0:c0 + 128],
                            hidH[k][:, j * 192:(j + 1) * 192],
                            start=(n == 0), stop=(n == 2 * KPA - 1))
                        n += 1
                # merge: cstg[slot-major] = psL[slot-major] + psH[parity]
                # (DVE may read only one operand from PSUM: copy, then add)
                nc.vector.tensor_copy(cstg[:, dp * NT:(dp + 1) * NT],
                                      psL[:, :NT])
                for nn in range(NCLS):
                    scol = 192 * (nn % 2) + 64 * (nn // 2)
                    cs = cstg[:, dp * NT + nn * 64:dp * NT + (nn + 1) * 64]
                    nc.vector.tensor_add(cs, cs, psH[:, scol:scol + 64])
            nc.sync.dma_start(cpartT_d[:], cstg[:])

            patch_chunk(1)
            patch_chunk(2)
            patch_chunk(3)

    nc.compile()
    return nc


def _sigmoid(x):
    out = np.empty_like(x)
    pos = x >= 0
    out[pos] = 1.0 / (1.0 + np.exp(-x[pos]))
    ex = np.exp(x[~pos])
    out[~pos] = ex / (1.0 + ex)
    return out


def kernel(x, patch_w1, patch_b1, patch_w2, patch_b2, gate_delta,
           atom_in_w, atom_in_b, atom_out_w, atom_out_b):
    x = np.asarray(x, dtype=np.float32)
    patch_w1 = np.asarray(patch_w1, dtype=np.float32)
    patch_b1 = np.asarray(patch_b1, dtype=np.float32)
    patch_w2 = np.asarray(patch_w2, dtype=np.float32)
    patch_b2 = np.asarray(patch_b2, dtype=np.float32)
    gate_delta = np.asarray(gate_delta, dtype=np.float32)
    atom_in_w = np.asarray(atom_in_w, dtype=np.float32)
    atom_in_b = np.asarray(atom_in_b, dtype=np.float32)
    atom_out_w = np.asarray(atom_out_w, dtype=np.float32)
    atom_out_b = np.asarray(atom_out_b, dtype=np.float32)

    bf = ml_dtypes.bfloat16

    # ---- host routing (tiny) ----
    cls3 = x[:, :NCLS, :]                                   # [B, 6, D]
    logits = np.einsum("bnd,nd->bn", cls3, gate_delta)      # [B, 6] f32
    choose_left = logits >= 0
    p_left = _sigmoid(logits)
    wgt = np.where(choose_left, p_left, 1.0 - p_left).astype(np.float32)
    keys = np.where(choose_left, LEFT_KEYS[None, :], RIGHT_KEYS[None, :])
    dst = (keys % NA).reshape(-1)                           # [B*6]
    wflat = wgt.reshape(-1)

    # token order on device: t = n*64 + b (slot-major)
    src_sm = (keys // NA).T.reshape(-1)                     # [384] slot-major
    left_sm = choose_left.T.reshape(-1)
    w_sm = wgt.T.reshape(-1)

    msrc = (src_sm[None, :] == np.arange(NA)[:, None]).astype(np.float32)
    mL = np.where(left_sm, w_sm, 0.0).astype(np.float32)
    mR = np.where(~left_sm, w_sm, 0.0).astype(np.float32)
    mall_rep = np.ascontiguousarray(np.broadcast_to(
        np.concatenate([msrc.reshape(-1), mL, mR]).reshape(1, (NA + 2) * NT),
        (128, (NA + 2) * NT))).astype(bf)

    # ---- replicated tensors (partition-major packed) ----
    # clsT[p, d*NT + t] = cls_sm[t, d*128+p],  t slot-major (n*64+b)
    cls_sm = np.ascontiguousarray(cls3.transpose(1, 0, 2)).reshape(NT, D)
    clsT = np.ascontiguousarray(
        cls_sm.reshape(NT, DT, 128).transpose(2, 1, 0)
    ).reshape(128, DT * NT).astype(bf)
    # w1s[p, h*768 + d*128 + m] = patch_w1[h*128+m, d*128+p]
    w1s = np.ascontiguousarray(
        patch_w1.reshape(HT, 128, DT, 128).transpose(3, 0, 2, 1)
    ).reshape(128, HT * DT * 128).astype(bf)
    b1T = np.ascontiguousarray(patch_b1.reshape(HT, 128).T)
    # w2T[p, h*D + dp*128 + m] = patch_w2[dp*128+m, h*128+p]
    w2T = np.ascontiguousarray(
        patch_w2.reshape(DT, 128, HT, 128).transpose(3, 2, 0, 1)
    ).reshape(128, HT * D).astype(bf)

    # ---- per-core tensors ----
    patch = x[:, NCLS:, :].reshape(NCORES, TPC, D)
    # xT[p, ci*DT*CW + d*CW + t] = patch[c][ci*CW+t, d*128+p]
    xT_all = np.ascontiguousarray(
        patch.reshape(NCORES, NCH, CW, DT, 128).transpose(0, 4, 1, 3, 2)
    ).reshape(NCORES, 128, NCH * DT * CW).astype(bf)

    ain2_all, ainbT_all, aoutT_all = [], [], []
    for c in range(NCORES):
        hsl = slice(HSH * c, HSH * (c + 1))
        # ain2[a*3+k][p, d*128+m] = atom_in_w[a, hsl0+k*128+m, d*128+p]
        ain2 = np.ascontiguousarray(
            atom_in_w[:, hsl, :].reshape(NA, KPA, 128, DT, 128)
            .transpose(0, 1, 4, 3, 2)).reshape(NG, 128, DT * 128).astype(bf)
        ain2_all.append(ain2)
        ainbT_all.append(np.ascontiguousarray(
            atom_in_b[:, hsl].reshape(NG, 128).T))
        # aoutF[p, a*KPA*D + k*D + dp*128 + m]
        #   = atom_out_w[a, dp*128+m, hsl0+k*128+p]
        aoutF = np.ascontiguousarray(
            atom_out_w[:, :, hsl].reshape(NA, DT, 128, KPA, 128)
            .transpose(4, 0, 3, 1, 2)).reshape(128, NA * KPA * D).astype(bf)
        aoutT_all.append(aoutF)

    in_maps = []
    for c in range(NCORES):
        in_maps.append({
            "xT": xT_all[c], "w1s": w1s, "b1T": b1T, "w2T": w2T,
            "clsT": clsT, "ain2": ain2_all[c], "ainbT": ainbT_all[c],
            "aoutF": aoutT_all[c], "mall": mall_rep,
        })

    nc = _CACHE.get("nc")
    if nc is None:
        nc = _build_program()
        _CACHE["nc"] = nc

    res = run_bass_kernel_spmd(nc, in_maps, core_ids=list(range(NCORES)))
    global LAST_RESULTS
    LAST_RESULTS = res

    # ---- host gather ----
    patch_out = np.empty((B, P, D), dtype=np.float32)
    for c in range(NCORES):
        # pout2[ci][p, d*CW+t] -> [D, TPC]
        po = res.results[c]["pout2"].astype(np.float32)     # [NCH,128,DT*CW]
        po = po.reshape(NCH, 128, DT, CW).transpose(2, 1, 0, 3).reshape(D, TPC)
        patch_out[BPC * c:BPC * (c + 1)] = (
            po.T + patch_b2[None, :]).reshape(BPC, P, D)

    cpart = np.zeros((128, DT * NT), dtype=np.float32)
    for c in range(NCORES):
        cpart += res.results[c]["cpartT"]
    cls_sm_out = cpart.reshape(128, DT, NT).transpose(1, 0, 2).reshape(D, NT).T
    # slot-major [n*64+b] -> [b, n]
    cls_out = np.ascontiguousarray(
        cls_sm_out.reshape(NCLS, B, D).transpose(1, 0, 2))
    cls_out += (wflat[:, None] * atom_out_b[dst, :]).reshape(B, NCLS, D)

    return np.concatenate([cls_out, patch_out], axis=1)


# revision 52
# speedup vs baseline: 1.1747x; 1.0013x over previous
"""Trainium2 Bass kernel for nn_Mlp_moe: dense patch-token MLP + top-1 gated
atom (expert) routing for 6 CLS task tokens.

Sharding over 8 NeuronCores:
  - Patch MLP: data-parallel over batch B=64 -> 8 batches (1568 patch tokens)
    per core. MLP weights replicated (SBUF-resident, bf16).
  - Atom/CLS part: hidden dim H=3072 sharded 8-way (384 per core); every core
    processes all 384 CLS tokens on its H-shard and emits a partial output
    summed on the host. Routing (gate logits/sigmoid/top-1) is computed on
    the host (O(B*6*D), negligible) and shipped as mask vectors.

Structure of the routing: token at slot n either goes left (src=n//2,
dst=3+n%2) or right (src=3+n%2, dst=n//2) with weight w. The in-GEMM is
computed for all 5 atoms (it doubles as DMA-latency cover at kernel start);
the chosen hid is selected with masks; the out-GEMM only computes the two
dst candidates per token (2/5 of the all-atom cost):
  psumL[tok in slots 2a,2a+1] += aout[a]    @ (sel * w*1[right])   a=0,1,2
  psumH[tok, parity j blocks] += aout[3+j]  @ (sel * w*1[left])    j=0,1

DMA: inputs are packed host-side into partition-major slices and issued in
compute-need order, alternating between the two fast HWDGE queues (sync /
scalar); per-(a,k) ain slices and per-4-h-tile w1/w2 slices let compute
start as soon as the first ~1MB lands instead of after whole tensors.
Patch outputs are written in bf16 (host upcasts and adds the bias).
"""

import numpy as np
import ml_dtypes

import concourse.bass as bass
import concourse.bacc as bacc
import concourse.mybir as mybir
from concourse import tile
from concourse.bass_utils import run_bass_kernel_spmd

NCORES = 8
B, NCLS, P, D, H = 64, 6, 196, 768, 3072
NA = 5
HSH = H // NCORES            # 384: per-core atom hidden shard
BPC = B // NCORES            # 8 batches per core
TPC = BPC * P                # 1568 patch tokens per core
NT = B * NCLS                # 384 cls tokens
DT = D // 128                # 6 d-tiles
HT = H // 128                # 24 h-tiles
KPA = HSH // 128             # 3 h-shard tiles per atom
NG = NA * KPA                # 15 (atom, k) in-GEMM groups
CW = 392
NCH = 4
WP = 4                       # h-tiles per w1/w2 DMA piece
NWP = HT // WP               # 6 pieces

LEFT_KEYS = np.array([3, 4, 8, 9, 13, 14], dtype=np.int64)
RIGHT_KEYS = np.array([15, 20, 16, 21, 17, 22], dtype=np.int64)

BF16 = mybir.dt.bfloat16
F32 = mybir.dt.float32
AF = mybir.ActivationFunctionType

_CACHE = {}
LAST_RESULTS = None  # BassKernelResults of the most recent run (for profiling)


def _build_program():
    nc = bacc.Bacc(None, target_bir_lowering=False, debug=False,
                   num_devices=NCORES)

    # ---- DRAM inputs (partition-major packed, see host layouts below) ----
    clsT_d = nc.dram_tensor("clsT", [128, DT * NT], BF16,
                            kind="ExternalInput")
    ainbT_d = nc.dram_tensor("ainbT", [128, NG], F32, kind="ExternalInput")
    b1T_d = nc.dram_tensor("b1T", [128, HT], F32, kind="ExternalInput")
    ain2_d = nc.dram_tensor("ain2", [NG, 128, DT * 128], BF16,
                            kind="ExternalInput")
    xT_d = nc.dram_tensor("xT", [128, NCH * DT * CW], BF16,
                          kind="ExternalInput")
    w1s_d = nc.dram_tensor("w1s", [128, HT * DT * 128], BF16,
                           kind="ExternalInput")
    w2T_d = nc.dram_tensor("w2T", [128, HT * D], BF16, kind="ExternalInput")
    mall_d = nc.dram_tensor("mall", [128, (NA + 2) * NT], BF16,
                            kind="ExternalInput")
    aoutF_d = nc.dram_tensor("aoutF", [128, NA * KPA * D], BF16,
                             kind="ExternalInput")
    # outputs: pout2[ci][p, d*CW + t] = patch_out_pre_bias[d*128+p, ci*CW+t]
    pout2_d = nc.dram_tensor("pout2", [NCH, 128, DT * CW], BF16,
                             kind="ExternalOutput")
    cpartT_d = nc.dram_tensor("cpartT", [128, DT * NT], F32,
                              kind="ExternalOutput")

    with tile.TileContext(nc) as tc:
        with (
            tc.tile_pool(name="w", bufs=1) as wp,
            tc.tile_pool(name="gat", bufs=1) as gp,
            tc.tile_pool(name="sel", bufs=1) as sp,
            tc.tile_pool(name="g1", bufs=24) as g1p,
            tc.tile_pool(name="ostg", bufs=1) as op,
            tc.tile_pool(name="cstg", bufs=1) as cp,
            tc.tile_pool(name="ps", bufs=8, space="PSUM") as pp,
        ):
            # ---- resident tiles ----
            clsT = wp.tile([128, DT * NT], BF16, tag="cls", name="cls")
            ainbT = wp.tile([128, NG], F32, tag="ainb", name="ainb")
            b1T = wp.tile([128, HT], F32, tag="b1", name="b1")
            ain2 = [wp.tile([128, DT * 128], BF16, tag=f"ain{g}",
                            name=f"ain{g}") for g in range(NG)]
            xs = [wp.tile([128, DT * CW], BF16, tag=f"x{c}", name=f"x{c}")
                  for c in range(NCH)]
            w1s = wp.tile([128, HT * DT * 128], BF16, tag="w1", name="w1")
            w2T = wp.tile([128, HT * D], BF16, tag="w2", name="w2")
            # masks combined into one tile: cols [0, NA*NT) = msrc per atom,
            # [NA*NT, (NA+1)*NT) = mL, [(NA+1)*NT, (NA+2)*NT) = mR
            mall = wp.tile([128, (NA + 2) * NT], BF16, tag="mall",
                           name="mall")
            aoutF = wp.tile([128, NA * KPA * D], BF16, tag="aoF",
                            name="aoF")

            # ---- DMA emission machinery ----
            # dma_start issues each wait on one of 8 round-robin completion
            # semaphore lanes (issue N blocks on completion of issue N-8), so
            # a long run of back-to-back dma_starts serializes the issuing
            # engine's queue.  Only the startup-critical transfers are issued
            # up front; the rest are threaded into the compute stream at
            # points where their lane-waits have long resolved.  Alternating
            # sync/scalar (the two HWDGE rings) keeps both rings pulling, and
            # FIFO-per-ring makes data land in need order.
            XW = DT * CW
            W1P = WP * DT * 128
            W2P = WP * D
            items = {}
            for j in range(3):  # cls in d-pair thirds
                c0, c1 = j * 2 * NT, (j + 1) * 2 * NT
                items[f"cls{j}"] = (clsT[:, c0:c1], clsT_d[:, c0:c1])
            items["ainb"] = (ainbT[:], ainbT_d[:])
            items["b1"] = (b1T[:], b1T_d[:])
            for g in range(NG):
                items[f"ain{g}"] = (ain2[g][:], ain2_d[g])
            for hf in range(2):
                c0 = hf * (XW // 2)
                items[f"x0_{hf}"] = (xs[0][:, c0:c0 + XW // 2],
                                     xT_d[:, c0:c0 + XW // 2])
            for c in range(1, NCH):
                items[f"x{c}"] = (xs[c][:], xT_d[:, c * XW:(c + 1) * XW])
            for j in range(NWP):
                items[f"w1p{j}"] = (w1s[:, j * W1P:(j + 1) * W1P],
                                    w1s_d[:, j * W1P:(j + 1) * W1P])
            for j in range(2):  # w2 in halves
                c0, c1 = j * 12 * D, (j + 1) * 12 * D
                items[f"w2p{j}"] = (w2T[:, c0:c1], w2T_d[:, c0:c1])
            items["mall"] = (mall[:], mall_d[:])
            AOH = NA * KPA * D // 2
            for j in range(2):  # aout in halves
                c0, c1 = j * AOH, (j + 1) * AOH if j else AOH
                items[f"aoF{j}"] = (aoutF[:, c0:c1], aoutF_d[:, c0:c1])

            dma_i = [0]

            def dma(*names, eng=None):
                for nm in names:
                    dst, src = items.pop(nm)
                    e = eng
                    if e is None:
                        e = nc.sync if dma_i[0] % 2 == 0 else nc.scalar
                        dma_i[0] += 1
                    e.dma_start(dst, src)

            # scalar (which also runs the gelus) gets ONLY the small
            # startup-critical transfers; its dma-issue chain ends by ~16us.
            # sync gets everything else up front in need order: its
            # semaphore-lane waits serialize harmlessly (no compute there),
            # and a lone HWDGE ring saturates HBM (~350GB/s measured).
            # x1..x3 ride at the end of the sync chain: the lane
            # serialization delays their issue past the startup-critical
            # window, so they never compete with cls/ain/w1 for HBM.
            dma("cls1", *[f"ain{g}" for g in range(0, NG, 2)],
                eng=nc.scalar)
            dma("cls0", "ain1", "cls2", "ainb", "b1",
                *[f"ain{g}" for g in range(3, NG, 2)],
                "x0_0", "x0_1", *[f"w1p{j}" for j in range(NWP)],
                "w2p0", "w2p1", "mall", "aoF0", "aoF1",
                "x1", "x2", "x3",
                eng=nc.sync)

            # ---- PE warm-up: dummy matmuls while the first DMAs land, so
            # the HAM clock-gate reaches 8/8 before real work starts ----
            warm = wp.tile([128, 512], BF16, tag="warm", name="warm")
            nc.vector.memset(warm[:], 0)
            wps = pp.tile([128, 512], F32, tag="ps", name="ps")
            # the HAM activity window is free-running, so ~2 windows (6.8us)
            # of PE busy guarantees the warm flip before real work starts
            NWARM = 12
            for i in range(NWARM):
                nc.tensor.matmul(wps[:, :512], warm[:, :128], warm[:],
                                 start=(i == 0), stop=(i == NWARM - 1))

            # ---- phase A: atom in-GEMM + gelu, all 5 atoms, (a,k)-major ---
            G = {}
            for g in range(NG):
                ps = pp.tile([128, 512], F32, tag="ps", name="ps")
                for d in range(DT):
                    nc.tensor.matmul(ps[:, :NT],
                                     ain2[g][:, d * 128:(d + 1) * 128],
                                     clsT[:, d * NT:(d + 1) * NT],
                                     start=(d == 0), stop=(d == DT - 1))
                gt = gp.tile([128, NT], BF16, tag=f"g{g}", name=f"g{g}")
                nc.scalar.activation(gt[:], ps[:, :NT], AF.Gelu,
                                     bias=ainbT[:, g:g + 1])
                G[g] = gt

            def phase_b():
                # ---- phase B (DVE): src-select, then dst-candidate masks
                hidL, hidH = [], []
                for k in range(KPA):
                    sel = sp.tile([128, NT], BF16, tag=f"sel{k}",
                                  name=f"sel{k}")
                    tmp = sp.tile([128, NT], BF16, tag="tmp", name="tmp")
                    nc.vector.tensor_mul(sel[:], G[0 * KPA + k][:],
                                         mall[:, :NT])
                    for a in range(1, NA):
                        nc.vector.tensor_mul(tmp[:], G[a * KPA + k][:],
                                             mall[:, a * NT:(a + 1) * NT])
                        nc.vector.tensor_add(sel[:], sel[:], tmp[:])
                    hl = sp.tile([128, NT], BF16, tag=f"hL{k}",
                                 name=f"hL{k}")
                    nc.vector.tensor_mul(
                        hl[:], sel[:],
                        mall[:, (NA + 1) * NT:(NA + 2) * NT])
                    hh = sp.tile([128, NT], BF16, tag=f"hH{k}",
                                 name=f"hH{k}")
                    for n in range(NCLS):
                        dcol = 192 * (n % 2) + 64 * (n // 2)
                        mc = NA * NT + n * 64
                        nc.vector.tensor_mul(hh[:, dcol:dcol + 64],
                                             sel[:, n * 64:(n + 1) * 64],
                                             mall[:, mc:mc + 64])
                    hidL.append(hl)
                    hidH.append(hh)
                return hidL, hidH

            def patch_chunk(ci, mid=None, g2_split=False):
                xa = xs[ci]
                g1s = []
                for h in range(HT):
                    ps = pp.tile([128, 512], F32, tag="ps", name="ps")
                    for d in range(DT):
                        nc.tensor.matmul(
                            ps[:, :CW],
                            w1s[:, h * DT * 128 + d * 128:
                                h * DT * 128 + (d + 1) * 128],
                            xa[:, d * CW:(d + 1) * CW],
                            start=(d == 0), stop=(d == DT - 1))
                    g1 = g1p.tile([128, CW], BF16, tag="g1", name="g1")
                    nc.scalar.activation(g1[:], ps[:, :CW], AF.Gelu,
                                         bias=b1T[:, h:h + 1])
                    g1s.append(g1)
                mid_out = mid() if mid is not None else None
                stg = op.tile([128, DT * CW], BF16, tag="ostg", name="ostg")
                if g2_split:
                    # two h-passes over all dp banks: h16-23's w2 slices
                    # (still in flight at chunk-0 time) are only needed for
                    # the second pass, ~16us later than dp-major order
                    psd = [pp.tile([128, 512], F32, tag="ps", name="ps")
                           for _ in range(DT)]
                    for h0, h1 in ((0, 16), (16, HT)):
                        for dp in range(DT):
                            for h in range(h0, h1):
                                nc.tensor.matmul(
                                    psd[dp][:, :CW],
                                    w2T[:, h * D + dp * 128:
                                        h * D + (dp + 1) * 128],
                                    g1s[h][:, :CW],
                                    start=(h == 0), stop=(h == HT - 1),
                                    skip_group_check=True)
                    for dp in range(DT):
                        nc.vector.tensor_copy(stg[:, dp * CW:(dp + 1) * CW],
                                              psd[dp][:, :CW])
                else:
                    for dp in range(DT):
                        ps = pp.tile([128, 512], F32, tag="ps", name="ps")
                        for h in range(HT):
                            nc.tensor.matmul(
                                ps[:, :CW],
                                w2T[:, h * D + dp * 128:h * D + (dp + 1) * 128],
                                g1s[h][:, :CW],
                                start=(h == 0), stop=(h == HT - 1))
                        nc.vector.tensor_copy(stg[:, dp * CW:(dp + 1) * CW],
                                              ps[:, :CW])
                        if ci == NCH - 1:
                            # tail chunk: per-dp DMAs on the fast queues
                            eng = nc.sync if dp % 2 == 0 else nc.scalar
                            eng.dma_start(
                                pout2_d[ci][:, dp * CW:(dp + 1) * CW],
                                stg[:, dp * CW:(dp + 1) * CW])
                if ci != NCH - 1:
                    nc.gpsimd.dma_start(pout2_d[ci], stg[:])
                return mid_out

            # ---- patch chunk 0, then atom out-GEMM, then chunks 1..3 ----
            hidL, hidH = patch_chunk(0, mid=phase_b, g2_split=True)

            cstg = cp.tile([128, DT * NT], F32, tag="cstg", name="cstg")
            for dp in range(DT):
                psL = pp.tile([128, 512], F32, tag="ps", name="ps")
                n = 0
                for a in range(3):
                    for k in range(KPA):
                        c0 = a * KPA * D + k * D + dp * 128
                        nc.tensor.matmul(
                            psL[:, a * 128:(a + 1) * 128],
                            aoutF[:, c0:c0 + 128],
                            hidL[k][:, a * 128:(a + 1) * 128],
                            start=(n == 0), stop=(n == 3 * KPA - 1))
                        n += 1
                psH = pp.tile([128, 512], F32, tag="ps", name="ps")
                n = 0
                for j in range(2):
                    for k in range(KPA):
                        c0 = (3 + j) * KPA * D + k * D + dp * 128
                        nc.tensor.matmul(
                            psH[:, j * 192:(j + 1) * 192],
                            aoutF[:, c0:c0 + 128],
                            hidH[k][:, j * 192:(j + 1) * 192],
                            start=(n == 0), stop=(n == 2 * KPA - 1))
                        n += 1
                # merge: cstg[slot-major] = psL[slot-major] + psH[parity]
                # (DVE may read only one operand from PSUM: copy, then add)
                nc.vector.tensor_copy(cstg[:, dp * NT:(dp + 1) * NT],
                                      psL[:, :NT])
                for nn in range(NCLS):
                    scol = 192 * (nn % 2) + 64 * (nn // 2)
                    cs = cstg[:, dp * NT + nn * 64:dp * NT + (nn + 1) * 64]
                    nc.vector.tensor_add(cs, cs, psH[:, scol:scol + 64])
            nc.sync.dma_start(cpartT_d[:], cstg[:])

            patch_chunk(1)
            patch_chunk(2)
            patch_chunk(3)

    nc.compile()
    return nc


def _sigmoid(x):
    out = np.empty_like(x)
    pos = x >= 0
    out[pos] = 1.0 / (1.0 + np.exp(-x[pos]))
    ex = np.exp(x[~pos])
    out[~pos] = ex / (1.0 + ex)
    return out


def kernel(x, patch_w1, patch_b1, patch_w2, patch_b2, gate_delta,
           atom_in_w, atom_in_b, atom_out_w, atom_out_b):
    x = np.asarray(x, dtype=np.float32)
    patch_w1 = np.asarray(patch_w1, dtype=np.float32)
    patch_b1 = np.asarray(patch_b1, dtype=np.float32)
    patch_w2 = np.asarray(patch_w2, dtype=np.float32)
    patch_b2 = np.asarray(patch_b2, dtype=np.float32)
    gate_delta = np.asarray(gate_delta, dtype=np.float32)
    atom_in_w = np.asarray(atom_in_w, dtype=np.float32)
    atom_in_b = np.asarray(atom_in_b, dtype=np.float32)
    atom_out_w = np.asarray(atom_out_w, dtype=np.float32)
    atom_out_b = np.asarray(atom_out_b, dtype=np.float32)

    bf = ml_dtypes.bfloat16

    # ---- host routing (tiny) ----
    cls3 = x[:, :NCLS, :]                                   # [B, 6, D]
    logits = np.einsum("bnd,nd->bn", cls3, gate_delta)      # [B, 6] f32
    choose_left = logits >= 0
    p_left = _sigmoid(logits)
    wgt = np.where(choose_left, p_left, 1.0 - p_left).astype(np.float32)
    keys = np.where(choose_left, LEFT_KEYS[None, :], RIGHT_KEYS[None, :])
    dst = (keys % NA).reshape(-1)                           # [B*6]
    wflat = wgt.reshape(-1)

    # token order on device: t = n*64 + b (slot-major)
    src_sm = (keys // NA).T.reshape(-1)                     # [384] slot-major
    left_sm = choose_left.T.reshape(-1)
    w_sm = wgt.T.reshape(-1)

    msrc = (src_sm[None, :] == np.arange(NA)[:, None]).astype(np.float32)
    mL = np.where(left_sm, w_sm, 0.0).astype(np.float32)
    mR = np.where(~left_sm, w_sm, 0.0).astype(np.float32)
    mall_rep = np.ascontiguousarray(np.broadcast_to(
        np.concatenate([msrc.reshape(-1), mL, mR]).reshape(1, (NA + 2) * NT),
        (128, (NA + 2) * NT))).astype(bf)

    # ---- replicated tensors (partition-major packed) ----
    # clsT[p, d*NT + t] = cls_sm[t, d*128+p],  t slot-major (n*64+b)
    cls_sm = np.ascontiguousarray(cls3.transpose(1, 0, 2)).reshape(NT, D)
    clsT = np.ascontiguousarray(
        cls_sm.reshape(NT, DT, 128).transpose(2, 1, 0)
    ).reshape(128, DT * NT).astype(bf)
    # w1s[p, h*768 + d*128 + m] = patch_w1[h*128+m, d*128+p]
    w1s = np.ascontiguousarray(
        patch_w1.reshape(HT, 128, DT, 128).transpose(3, 0, 2, 1)
    ).reshape(128, HT * DT * 128).astype(bf)
    b1T = np.ascontiguousarray(patch_b1.reshape(HT, 128).T)
    # w2T[p, h*D + dp*128 + m] = patch_w2[dp*128+m, h*128+p]
    w2T = np.ascontiguousarray(
        patch_w2.reshape(DT, 128, HT, 128).transpose(3, 2, 0, 1)
    ).reshape(128, HT * D).astype(bf)

    # ---- per-core tensors ----
    patch = x[:, NCLS:, :].reshape(NCORES, TPC, D)
    # xT[p, ci*DT*CW + d*CW + t] = patch[c][ci*CW+t, d*128+p]
    xT_all = np.ascontiguousarray(
        patch.reshape(NCORES, NCH, CW, DT, 128).transpose(0, 4, 1, 3, 2)
    ).reshape(NCORES, 128, NCH * DT * CW).astype(bf)

    ain2_all, ainbT_all, aoutT_all = [], [], []
    for c in range(NCORES):
        hsl = slice(HSH * c, HSH * (c + 1))
        # ain2[a*3+k][p, d*128+m] = atom_in_w[a, hsl0+k*128+m, d*128+p]
        ain2 = np.ascontiguousarray(
            atom_in_w[:, hsl, :].reshape(NA, KPA, 128, DT, 128)
            .transpose(0, 1, 4, 3, 2)).reshape(NG, 128, DT * 128).astype(bf)
        ain2_all.append(ain2)
        ainbT_all.append(np.ascontiguousarray(
            atom_in_b[:, hsl].reshape(NG, 128).T))
        # aoutF[p, a*KPA*D + k*D + dp*128 + m]
        #   = atom_out_w[a, dp*128+m, hsl0+k*128+p]
        aoutF = np.ascontiguousarray(
            atom_out_w[:, :, hsl].reshape(NA, DT, 128, KPA, 128)
            .transpose(4, 0, 3, 1, 2)).reshape(128, NA * KPA * D).astype(bf)
        aoutT_all.append(aoutF)

    in_maps = []
    for c in range(NCORES):
        in_maps.append({
            "xT": xT_all[c], "w1s": w1s, "b1T": b1T, "w2T": w2T,
            "clsT": clsT, "ain2": ain2_all[c], "ainbT": ainbT_all[c],
            "aoutF": aoutT_all[c], "mall": mall_rep,
        })

    nc = _CACHE.get("nc")
    if nc is None:
        nc = _build_program()
        _CACHE["nc"] = nc

    res = run_bass_kernel_spmd(nc, in_maps, core_ids=list(range(NCORES)))
    global LAST_RESULTS
    LAST_RESULTS = res

    # ---- host gather ----
    patch_out = np.empty((B, P, D), dtype=np.float32)
    for c in range(NCORES):
        # pout2[ci][p, d*CW+t] -> [D, TPC]
        po = res.results[c]["pout2"].astype(np.float32)     # [NCH,128,DT*CW]
        po = po.reshape(NCH, 128, DT, CW).transpose(2, 1, 0, 3).reshape(D, TPC)
        patch_out[BPC * c:BPC * (c + 1)] = (
            po.T + patch_b2[None, :]).reshape(BPC, P, D)

    cpart = np.zeros((128, DT * NT), dtype=np.float32)
    for c in range(NCORES):
        cpart += res.results[c]["cpartT"]
    cls_sm_out = cpart.reshape(128, DT, NT).transpose(1, 0, 2).reshape(D, NT).T
    # slot-major [n*64+b] -> [b, n]
    cls_out = np.ascontiguousarray(
        cls_sm_out.reshape(NCLS, B, D).transpose(1, 0, 2))
    cls_out += (wflat[:, None] * atom_out_b[dst, :]).reshape(B, NCLS, D)

    return np.concatenate([cls_out, patch_out], axis=1)
